# revision 28
# baseline (speedup 1.0000x reference)
"""Channel cross-attention kernel for Trainium2 (8 NeuronCores).

Math (exact restructuring of the reference):
    xf = x.reshape(B, C, N)
    q = wq xf + bq;  k = wk xf + bk;  v = wv xf + bv
    energy = q k^T = wq G wk^T + (wq sx) bk^T + bq (wk sx)^T + N bq bk^T
        where G = xf xf^T (C x C Gram), sx = xf @ 1 (row sums)
    att = softmax(energy / sqrt(N))
    out = att v + xf = (att wv) xf + (att bv) 1^T + xf = M xf + r 1^T + xf

Sharding: 8 cores, core i handles sample b=i//2, spatial half h=i%2.
Each core computes G over the FULL sample (redundantly within the pair, no
cross-core communication) and produces its own spatial half of the output.

Precision: x streams in bf16 (halves HBM read), out streams back bf16
(halves HBM write; host upcasts), everything between Gram and the output
matmul runs in f32(r). The Gram uses the symmetry G = G^T: the second
row-block only streams cols 128:258 and the missing 128x128 block is
reconstructed by one PE transpose.

Host prep per core: xt = xf[b].T (N, C+2: data, ones, pad) in bf16, rows
ordered own-spatial-half first. The Gram phase streams all rows; the own
half stays resident in SBUF and is transposed back on-chip for the output
phase.
"""

import os
import sys

for _p in ("/opt/trn_rl_repo", "/root/.axon_site/_ro/trn_rl_repo"):
    if os.path.isdir(_p) and _p not in sys.path:
        sys.path.append(_p)

import numpy as np
import ml_dtypes

# ---- problem constants (hardcoded; must match setup_inputs) ----
B, C, W, H = 4, 256, 128, 128
N = W * H            # 16384
HALF = N // 2        # 8192
P = 128              # partitions
NCORES = 8
SQRT_N = float(np.sqrt(N))   # 128.0
XT_COLS_D = C + 2            # xt DRAM row: 256 data cols, ones col, zero pad

GRAM_DT = "bf16"     # informational (printed by test harness)
MM_DT = "bf16"

_BUILD_CACHE = {}
LAST_RESULT = None   # BassKernelResults of the most recent run (for test harness)


def _build():
    import concourse.bacc as bacc
    import concourse.mybir as mybir
    import concourse.tile as tile
    from concourse.bass import MemorySpace
    from concourse.masks import make_identity

    f32 = mybir.dt.float32
    f32r = mybir.dt.float32r
    bf16 = mybir.dt.bfloat16

    nc = bacc.Bacc("TRN2", target_bir_lowering=False)

    xt_d = nc.dram_tensor("xt", (N, XT_COLS_D), bf16, kind="ExternalInput")
    # natural-layout own-half cols 4096:8192 (phase C chunks 2-3): loaded
    # directly instead of PE-transposing, filling the post-input DMA idle gap
    xn_d = nc.dram_tensor("xn", (C, HALF // 2), bf16, kind="ExternalInput")
    wq_d = nc.dram_tensor("wq", (C, C), f32r, kind="ExternalInput")
    bq_d = nc.dram_tensor("bq", (C,), f32r, kind="ExternalInput")
    wk_d = nc.dram_tensor("wk", (C, C), f32r, kind="ExternalInput")
    bk_d = nc.dram_tensor("bk", (C,), f32r, kind="ExternalInput")
    wv_d = nc.dram_tensor("wv", (C, C), bf16, kind="ExternalInput")
    bv_d = nc.dram_tensor("bv", (C,), bf16, kind="ExternalInput")
    out_d = nc.dram_tensor("out", (C, HALF), bf16, kind="ExternalOutput")

    xt_ap = xt_d.ap()
    out_ap = out_d.ap()

    NT = N // P          # 128 n-tiles for the Gram phase
    CH_T = 16            # n-tiles per SBUF tile
    NCHUNK = NT // CH_T  # 8 chunks
    Exp = mybir.ActivationFunctionType.Exp
    Copy = mybir.ActivationFunctionType.Copy
    Ident = mybir.ActivationFunctionType.Identity

    with tile.TileContext(nc) as tc:
        with (
            tc.tile_pool(name="singles", bufs=1) as singles,
            tc.tile_pool(name="work", bufs=2) as work,
        ):
            # ---------- constants ----------
            identity = singles.tile([P, P], f32, tag="ident", name="ident")
            make_identity(nc, identity)
            identity_r = singles.tile([P, P], bf16, tag="identr", name="identr")
            nc.vector.tensor_copy(out=identity_r, in_=identity)
            identity_fr = singles.tile([P, P], f32r, tag="identfr", name="identfr")
            nc.vector.tensor_copy(out=identity_fr, in_=identity)
            # warm the ACT Exp table early so phase B's exp doesn't pay the
            # ~1.3us table load on the critical path
            warm = singles.tile([1, 2], f32, tag="warm", name="warm")
            nc.vector.memset(warm, 0.0)
            nc.scalar.activation(out=warm, in_=warm, func=Exp,
                                 bias=0.0, scale=1.0)

            # weights natural layout (2 row-tiles each), f32r so every
            # phase-B matmul runs at full PE rate
            wv_sb = []
            wq_sb = []
            wk_sb = []
            for j in range(2):
                t = singles.tile([P, C], bf16, tag=f"wv{j}", name=f"wv{j}")
                nc.gpsimd.dma_start(out=t, in_=wv_d.ap()[j * P:(j + 1) * P, :])
                wv_sb.append(t)
                t = singles.tile([P, C], f32r, tag=f"wq{j}", name=f"wq{j}")
                nc.gpsimd.dma_start(out=t, in_=wq_d.ap()[j * P:(j + 1) * P, :])
                wq_sb.append(t)
                t = singles.tile([P, C], f32r, tag=f"wk{j}", name=f"wk{j}")
                nc.gpsimd.dma_start(out=t, in_=wk_d.ap()[j * P:(j + 1) * P, :])
                wk_sb.append(t)

            bq_row = singles.tile([1, C], f32r, tag="bqr", name="bqr")
            nc.gpsimd.dma_start(out=bq_row, in_=bq_d.ap().unsqueeze(0))
            bk_row = singles.tile([1, C], f32r, tag="bkr", name="bkr")
            nc.gpsimd.dma_start(out=bk_row, in_=bk_d.ap().unsqueeze(0))
            bkN_row = singles.tile([1, C], f32r, tag="bknr", name="bknr")
            nc.gpsimd.dma_start(out=bkN_row, in_=bk_d.ap().unsqueeze(0))
            nc.vector.tensor_scalar_mul(bkN_row, bkN_row, float(N))
            bv_col = []
            for j in range(2):
                t = singles.tile([P, 1], bf16, tag=f"bv{j}", name=f"bv{j}")
                nc.gpsimd.dma_start(out=t, in_=bv_d.ap()[j * P:(j + 1) * P].unsqueeze(1))
                bv_col.append(t)

            # transposed weights wqT[e][:, c] = wq[c, e], wkT likewise
            wqT_sb = [singles.tile([P, C], f32r, tag=f"wqT{j}", name=f"wqT{j}") for j in range(2)]
            wkT_sb = [singles.tile([P, C], f32r, tag=f"wkT{j}", name=f"wkT{j}") for j in range(2)]

            gsb = [singles.tile([P, C + 2], f32r, tag=f"gsb{m}", name=f"gsb{m}") for m in range(2)]
            t1sb = [singles.tile([P, C], f32r, tag=f"t1sb{m}", name=f"t1sb{m}") for m in range(2)]
            attT_sb = [singles.tile([P, C], bf16, tag=f"attT{m}", name=f"attT{m}") for m in range(2)]
            mt_sb = [singles.tile([P, C], bf16, tag=f"mt{m}", name=f"mt{m}") for m in range(2)]
            r_sb = [singles.tile([P, 1], f32, tag=f"r{m}", name=f"r{m}") for m in range(2)]
            sq_row = singles.tile([1, C], f32r, tag="sqr", name="sqr")
            sk_row = singles.tile([1, C], f32r, tag="skr", name="skr")
            ssum_sb = [singles.tile([P, 1], f32, tag=f"ssum{m}", name=f"ssum{m}") for m in range(2)]
            rs_sb = [singles.tile([P, 1], f32, tag=f"rs{m}", name=f"rs{m}") for m in range(2)]

            # ---------- phase A: Gram accumulation ----------
            # Symmetric Gram: row-block 0 streams all 258 cols; row-block 1
            # streams only cols 128:258 (G10 = G01^T is reconstructed by one
            # PE transpose afterwards). Own-half chunks are each PE-transposed
            # back to natural layout (xn) right after their DMA lands.
            NRES = NCHUNK // 2
            CW = CH_T * P     # chunk width in spatial cols (2048)
            TB = 4            # transposes batched per PSUM bank
            CH_T_S = 8        # n-tiles per streamed (non-resident) SBUF tile
            M1_LO, M1_W = P, XT_COLS_D - P   # cols 128:258 (130 wide)
            NTRES = NRES // 2  # chunks transposed on-chip (rest DMA'd via xn_d)
            xn = [singles.tile([P, HALF], bf16, tag=f"xn{m}",
                               name=f"xn{m}") for m in range(2)]
            with (
                tc.tile_pool(name="psg", bufs=1, space=MemorySpace.PSUM) as psg,
                tc.tile_pool(name="psct", bufs=5, space=MemorySpace.PSUM) as psct,
                tc.tile_pool(name="xtrp", bufs=3) as xtrp,
                tc.tile_pool(name="xtp", bufs=5) as xtp,
            ):
                g_ps0 = psg.tile([P, C + 2], f32, tag="g0", name="g0")
                g_ps1 = psg.tile([P, M1_W], f32, tag="g1", name="g1")
                # interleave own/streamed chunks so PE is never starved and
                # the DMA queue never runs dry
                jobs = [("own", 0), ("str", 0), ("own", 1), ("str", 1),
                        ("str", 2), ("own", 2), ("str", 3), ("str", 4),
                        ("own", 3), ("str", 5), ("str", 6), ("str", 7)]
                wjobs = [(wsrc, wdst, jj, ii)
                         for (wsrc, wdst) in ((wq_sb, wqT_sb), (wk_sb, wkT_sb))
                         for jj in range(2) for ii in range(2)]
                nt = 0
                ncopy = 0
                for kind, ch in jobs:
                    if kind == "own":
                        ctiles = CH_T
                        xt = xtrp.tile([P, CH_T, XT_COLS_D], bf16, tag="xtr",
                                       name="xtr")
                        if ch == 0:
                            # split the first chunk so PE starts ~4x sooner
                            QT = CH_T // 4
                            for q in range(4):
                                nc.sync.dma_start(
                                    out=xt[:, q * QT:(q + 1) * QT, :],
                                    in_=xt_ap[q * QT * P:(q + 1) * QT * P, :]
                                        .rearrange("(t p) c -> p t c", p=P),
                                )
                        else:
                            nc.sync.dma_start(
                                out=xt,
                                in_=xt_ap[ch * CH_T * P:(ch + 1) * CH_T * P, :]
                                    .rearrange("(t p) c -> p t c", p=P),
                            )
                    else:
                        ctiles = CH_T_S
                        xt = xtp.tile([P, CH_T_S, XT_COLS_D], bf16, tag="xt",
                                      name="xt")
                        row0 = NRES * CH_T * P + ch * CH_T_S * P
                        nc.sync.dma_start(
                            out=xt,
                            in_=xt_ap[row0:row0 + CH_T_S * P, :]
                                .rearrange("(t p) c -> p t c", p=P),
                        )
                    for t in range(ctiles):
                        nc.tensor.matmul(
                            g_ps0[:, 0:C + 2],
                            xt[:, t, 0:P],
                            xt[:, t, 0:C + 2],
                            start=(nt == 0), stop=(nt == NT - 1),
                        )
                        nc.tensor.matmul(
                            g_ps1[:, 0:M1_W],
                            xt[:, t, P:C],
                            xt[:, t, M1_LO:XT_COLS_D],
                            start=(nt == 0), stop=(nt == NT - 1),
                        )
                        nt += 1
                    if kind == "own" and ch < NTRES:
                        # transpose to natural layout; PSUM->SBUF copies
                        # alternate ACT/DVE
                        for m in range(2):
                            for tb in range(CH_T // TB):
                                tps = psct.tile([P, TB * P], bf16, tag="tps",
                                                name="tps")
                                for k in range(TB):
                                    t = tb * TB + k
                                    nc.tensor.transpose(
                                        tps[:, k * P:(k + 1) * P],
                                        xt[:, t, m * P:(m + 1) * P],
                                        identity_r)
                                dst = xn[m][:, ch * CW + tb * TB * P:
                                            ch * CW + (tb + 1) * TB * P]
                                if ncopy % 2 == 0:
                                    nc.vector.tensor_copy(out=dst, in_=tps)
                                else:
                                    nc.scalar.activation(out=dst, in_=tps,
                                                         func=Copy,
                                                         bias=0.0, scale=1.0)
                                ncopy += 1
                    elif kind == "str" and ch < len(wjobs):
                        wsrc, wdst, jj, ii = wjobs[ch]
                        ps = psct.tile([P, P], f32r, tag="wt", name="wt",
                                       bufs=1)
                        nc.tensor.transpose(
                            ps, wsrc[ii][:, jj * P:(jj + 1) * P], identity_fr)
                        nc.vector.tensor_copy(
                            out=wdst[jj][:, ii * P:(ii + 1) * P], in_=ps)

                # natural-layout chunks 2-3 arrive by DMA in the gap after
                # the Gram input finishes (queued behind all xt loads)
                for m in range(2):
                    nc.sync.dma_start(
                        out=xn[m][:, NTRES * CW:HALF],
                        in_=xn_d.ap()[m * P:(m + 1) * P, :])

                # G to SBUF: block row 0 fully (split across DVE/ACT; the
                # 128:258 slice lands first so the fixup transpose can start),
                # block row 1 cols 128:258 from PSUM via ACT
                nc.vector.tensor_copy(out=gsb[0][:, P:XT_COLS_D],
                                      in_=g_ps0[:, P:XT_COLS_D])
                nc.scalar.activation(out=gsb[0][:, 0:P], in_=g_ps0[:, 0:P],
                                     func=Copy, bias=0.0, scale=1.0)
                nc.scalar.activation(out=gsb[1][:, M1_LO:XT_COLS_D],
                                     in_=g_ps1, func=Copy, bias=0.0, scale=1.0)

            # ---------- phase B: energy^T, exp, M, r ----------
            # Everything is computed directly in the TRANSPOSED (d, c)
            # orientation (G is symmetric), so no PE<->DVE transpose
            # ping-pong. exp is taken without max-subtraction (energies
            # here are |e|/sqrt(N) < ~50, exp < 1e20, far from fp32
            # overflow); the 1/rowsum normalization is folded into the
            # phase-C output scale.
            with tc.tile_pool(name="psb", bufs=1, space=MemorySpace.PSUM) as psb:
                # G fixup: G10 = (G01)^T via one PE transpose
                tfix = psb.tile([P, P], f32r, tag="wt", name="tfix", bufs=1)
                nc.tensor.transpose(tfix, gsb[0][:, P:C], identity_fr)
                nc.vector.tensor_copy(out=gsb[1][:, 0:P], in_=tfix)

                # sq' = wq sx (row), sk' = wk sx (row) — early, they feed the
                # rank-1 energy terms; copies split ACT/DVE
                sq_ps = psb.tile([1, C], f32, tag="rps", name="sqp", bufs=2)
                for e in range(2):
                    nc.tensor.matmul(
                        sq_ps, gsb[e][:, C:C + 1], wqT_sb[e][:, 0:C],
                        start=(e == 0), stop=(e == 1))
                nc.scalar.activation(out=sq_row, in_=sq_ps, func=Copy,
                                     bias=0.0, scale=1.0)
                sk_ps = psb.tile([1, C], f32, tag="rps", name="skp", bufs=2)
                for e in range(2):
                    nc.tensor.matmul(
                        sk_ps, gsb[e][:, C:C + 1], wkT_sb[e][:, 0:C],
                        start=(e == 0), stop=(e == 1))
                nc.vector.tensor_copy(out=sk_row, in_=sk_ps)

                # T1q[a, c] = (G wq^T)[a, c]; e=0 contributions first so they
                # run while the G fixup completes
                t1_ps = [psb.tile([P, C], f32, tag="tmp", name=f"t1p{a}",
                                  bufs=3) for a in range(2)]
                for a in range(2):
                    nc.tensor.matmul(
                        t1_ps[a], gsb[0][:, a * P:(a + 1) * P],
                        wqT_sb[0][:, 0:C], start=True, stop=False)
                for a in range(2):
                    nc.tensor.matmul(
                        t1_ps[a], gsb[1][:, a * P:(a + 1) * P],
                        wqT_sb[1][:, 0:C], start=False, stop=True)
                nc.vector.tensor_copy(out=t1sb[0], in_=t1_ps[0])
                nc.scalar.activation(out=t1sb[1], in_=t1_ps[1], func=Copy,
                                     bias=0.0, scale=1.0)

                # energyT (raw, unscaled) per d-tile:
                # energyT[d, c] = (wk G wq^T)[d, c] + bk[d] sq'[c]
                #                 + sk'[d] bq[c] + N bk[d] bq[c]
                eT_ps = [psb.tile([P, C], f32, tag=f"eps{dt}", name=f"eps{dt}")
                         for dt in range(2)]
                for dt in range(2):
                    ds_ = (dt * P, (dt + 1) * P)
                    nc.tensor.matmul(
                        eT_ps[dt],
                        wkT_sb[0][:, dt * P:(dt + 1) * P],
                        t1sb[0][:, 0:C],
                        start=True, stop=False,
                    )
                    nc.tensor.matmul(eT_ps[dt], bkN_row[:, ds_[0]:ds_[1]],
                                     bq_row[:, 0:C], start=False, stop=False)
                    nc.tensor.matmul(eT_ps[dt], bk_row[:, ds_[0]:ds_[1]],
                                     sq_row[:, 0:C], start=False, stop=False)
                    nc.tensor.matmul(
                        eT_ps[dt],
                        wkT_sb[1][:, dt * P:(dt + 1) * P],
                        t1sb[1][:, 0:C],
                        start=False, stop=False,
                    )
                    nc.tensor.matmul(eT_ps[dt], sk_row[:, ds_[0]:ds_[1]],
                                     bq_row[:, 0:C], start=False, stop=True)
                    # attT (unnormalized): exp(energyT / sqrt(N))
                    nc.scalar.activation(
                        out=attT_sb[dt], in_=eT_ps[dt], func=Exp,
                        bias=0.0, scale=1.0 / SQRT_N)

                ones_col = singles.tile([P, 1], bf16, tag="ones", name="ones")
                nc.vector.memset(ones_col, 1.0)

                # row sums: ssum[c] = sum_d attT[d, c] (column via matmul)
                dg_bf = [work.tile([P, P], bf16, tag=f"dg{e}", name=f"dg{e}")
                         for e in range(2)]
                for ct in range(2):
                    ps = psb.tile([P, 1], f32, tag="rps", name="rps", bufs=2)
                    for d in range(2):
                        nc.tensor.matmul(
                            ps, attT_sb[d][:, ct * P:(ct + 1) * P], ones_col,
                            start=(d == 0), stop=(d == 1))
                    nc.vector.tensor_copy(out=ssum_sb[ct], in_=ps)
                    nc.vector.reciprocal(out=rs_sb[ct], in_=ssum_sb[ct])
                    # diag(ssum) as bf16 for the in-PSUM diagonal matmul
                    nc.vector.tensor_scalar_mul(dg_bf[ct], identity_r,
                                                ssum_sb[ct])

                # MT[e][:, c] = M~[c, e] = sum_d att~[c, d] wv[d, e]
                # (+ diag(ssum) added by a PE matmul so phase C's rs scale
                # yields M x + x); copies split DVE/ACT
                for e in range(2):
                    ps = psb.tile([P, C], f32, tag="tmp", name=f"mtp{e}",
                                  bufs=3)
                    for d in range(2):
                        nc.tensor.matmul(
                            ps,
                            wv_sb[d][:, e * P:(e + 1) * P],
                            attT_sb[d][:, 0:C],
                            start=(d == 0), stop=False,
                        )
                    nc.tensor.matmul(
                        ps[:, e * P:(e + 1) * P], dg_bf[e], identity_r,
                        start=False, stop=True)
                    if e == 0:
                        nc.vector.tensor_copy(out=mt_sb[e], in_=ps)
                    else:
                        nc.scalar.activation(out=mt_sb[e], in_=ps, func=Copy,
                                             bias=0.0, scale=1.0)

                # r[c] = rs[c] * sum_d att~[c, d] bv[d]
                for ct in range(2):
                    ps = psb.tile([P, 1], f32, tag="rps", name="rps", bufs=2)
                    for d in range(2):
                        nc.tensor.matmul(
                            ps, attT_sb[d][:, ct * P:(ct + 1) * P], bv_col[d],
                            start=(d == 0), stop=(d == 1))
                    nc.vector.tensor_copy(out=r_sb[ct], in_=ps)
                    nc.vector.tensor_mul(r_sb[ct], r_sb[ct], rs_sb[ct])

            # ---------- phase C: out = (M + I) x + r ----------
            MMW = 512         # matmul free width (one PSUM bank of f32)
            mult = mybir.AluOpType.mult
            add = mybir.AluOpType.add
            with (
                tc.tile_pool(name="psc", bufs=2, space=MemorySpace.PSUM) as psc,
                tc.tile_pool(name="outp", bufs=3) as outp,
            ):
                for ch in range(NRES):
                    for ct in range(2):
                        ot = outp.tile([P, CW], bf16, tag="ot", name="ot")
                        ps = psc.tile([P, CW], f32, tag="ops", name="ops")
                        for s in range(CW // MMW):
                            sl = slice(s * MMW, (s + 1) * MMW)
                            for e in range(2):
                                nc.tensor.matmul(
                                    ps[:, sl],
                                    mt_sb[e][:, ct * P:(ct + 1) * P],
                                    xn[e][:, ch * CW + s * MMW:
                                           ch * CW + (s + 1) * MMW],
                                    start=(e == 0), stop=(e == 1),
                                )
                        # ot = rs*psum + r (per-partition scale+bias), split
                        # ACT || DVE; the final tiles use finer slices so the
                        # last DMA starts (and ends) sooner
                        nslc = 4 if ch == NRES - 1 else 2
                        sw = CW // nslc
                        for s in range(nslc):
                            sl = slice(s * sw, (s + 1) * sw)
                            if s % 2 == 0:
                                nc.scalar.activation(
                                    out=ot[:, sl], in_=ps[:, sl], func=Ident,
                                    bias=r_sb[ct], scale=rs_sb[ct])
                            else:
                                nc.vector.tensor_scalar(
                                    out=ot[:, sl], in0=ps[:, sl],
                                    scalar1=rs_sb[ct], scalar2=r_sb[ct],
                                    op0=mult, op1=add)
                            nc.sync.dma_start(
                                out=out_ap[ct * P:(ct + 1) * P,
                                           ch * CW + s * sw:
                                           ch * CW + (s + 1) * sw],
                                in_=ot[:, sl])

    nc.compile()
    return nc


def _get_nc():
    key = "v2"
    if key not in _BUILD_CACHE:
        _BUILD_CACHE[key] = _build()
    return _BUILD_CACHE[key]


def kernel(x, wq, bq, wk, bk, wv, bv):
    global LAST_RESULT
    from concourse.bass_utils import run_bass_kernel_spmd

    nc = _get_nc()

    x = np.ascontiguousarray(np.asarray(x, dtype=np.float32))
    xf = x.reshape(B, C, N)
    wq = np.ascontiguousarray(np.asarray(wq, dtype=np.float32))
    wk = np.ascontiguousarray(np.asarray(wk, dtype=np.float32))
    wv = np.ascontiguousarray(np.asarray(wv, dtype=np.float32))
    bq = np.ascontiguousarray(np.asarray(bq, dtype=np.float32))
    bk = np.ascontiguousarray(np.asarray(bk, dtype=np.float32))
    bv = np.ascontiguousarray(np.asarray(bv, dtype=np.float32))

    in_maps = _make_in_maps(xf, wq, bq, wk, bk, wv, bv)

    res = run_bass_kernel_spmd(nc, in_maps, core_ids=list(range(NCORES)))
    LAST_RESULT = res

    out = np.empty((B, C, N), dtype=np.float32)
    for i in range(NCORES):
        b, h = i // 2, i % 2
        out[b, :, h * HALF:(h + 1) * HALF] = np.asarray(
            res.results[i]["out"]).astype(np.float32)
    return out.reshape(B, C, W, H)


# ---------------------------------------------------------------------------
# Dev-loop helpers (not used by the grading path)
# ---------------------------------------------------------------------------

def timeline_ns():
    """Cost-model simulated duration of one core's program (ns)."""
    from concourse.timeline_sim import TimelineSim
    nc = _get_nc()
    ts = TimelineSim(nc)
    return ts.simulate()


def _make_in_maps(xf, wq, bq, wk, bk, wv, bv):
    ones_pad = np.zeros((N, 2), dtype=np.float32)
    ones_pad[:, 0] = 1.0
    in_maps = []
    for i in range(NCORES):
        b, h = i // 2, i % 2
        xTb = np.concatenate([xf[b].T, ones_pad], axis=1)
        # own spatial half first: the kernel keeps the first NCHUNK/2 chunks
        # resident and derives its output columns from them
        xt = np.concatenate([xTb[h * HALF:(h + 1) * HALF],
                             xTb[(1 - h) * HALF:(2 - h) * HALF]],
                            axis=0).astype(ml_dtypes.bfloat16)
        xn = np.ascontiguousarray(
            xf[b][:, h * HALF + HALF // 2:(h + 1) * HALF]
        ).astype(ml_dtypes.bfloat16)
        in_maps.append({
            "xt": xt, "xn": xn,
            "wq": wq, "bq": bq, "wk": wk, "bk": bk,
            "wv": wv.astype(ml_dtypes.bfloat16),
            "bv": bv.astype(ml_dtypes.bfloat16),
        })
    return in_maps


# revision 34
# speedup vs baseline: 1.1428x; 1.1428x over previous
"""Channel cross-attention kernel for Trainium2 (8 NeuronCores).

Math (exact restructuring of the reference):
    xf = x.reshape(B, C, N)
    q = wq xf + bq;  k = wk xf + bk;  v = wv xf + bv
    energy = q k^T = wq G wk^T + (wq sx) bk^T + bq (wk sx)^T + N bq bk^T
        where G = xf xf^T (C x C Gram), sx = xf @ 1 (row sums)
    att = softmax(energy / sqrt(N))
    out = att v + xf = (att wv) xf + (att bv) 1^T + xf = M xf + r 1^T + xf

Sharding: 8 cores, core i handles sample b=i//2, spatial half h=i%2.
Each core computes G over the FULL sample (redundantly within the pair, no
cross-core communication) and produces its own spatial half of the output.

Precision: x streams in bf16 (halves HBM read), out streams back bf16
(halves HBM write; host upcasts), everything between Gram and the output
matmul runs in f32(r). The Gram uses the symmetry G = G^T: the second
row-block only streams cols 128:258 and the missing 128x128 block is
reconstructed by one PE transpose.

Host prep per core: xt = xf[b].T (N, C+2: data, ones, pad) in bf16, rows
ordered own-spatial-half first. The Gram phase streams all rows; the own
half stays resident in SBUF and is transposed back on-chip for the output
phase.
"""

import os
import sys

for _p in ("/opt/trn_rl_repo", "/root/.axon_site/_ro/trn_rl_repo"):
    if os.path.isdir(_p) and _p not in sys.path:
        sys.path.append(_p)

import numpy as np
import ml_dtypes

# ---- problem constants (hardcoded; must match setup_inputs) ----
B, C, W, H = 4, 256, 128, 128
N = W * H            # 16384
HALF = N // 2        # 8192
P = 128              # partitions
NCORES = 8
SQRT_N = float(np.sqrt(N))   # 128.0
XT_COLS_D = C + 2            # xt DRAM row: 256 data cols, ones col, zero pad

GRAM_DT = "bf16"     # informational (printed by test harness)
MM_DT = "bf16"

_BUILD_CACHE = {}
LAST_RESULT = None   # BassKernelResults of the most recent run (for test harness)


def _build():
    import concourse.bacc as bacc
    import concourse.mybir as mybir
    import concourse.tile as tile
    from concourse.bass import MemorySpace
    from concourse.masks import make_identity

    f32 = mybir.dt.float32
    f32r = mybir.dt.float32r
    bf16 = mybir.dt.bfloat16

    nc = bacc.Bacc("TRN2", target_bir_lowering=False)

    xt_d = nc.dram_tensor("xt", (N, XT_COLS_D), bf16, kind="ExternalInput")
    # natural-layout own-half cols 4096:8192 (phase C chunks 2-3): loaded
    # directly instead of PE-transposing, filling the post-input DMA idle gap
    xn_d = nc.dram_tensor("xn", (C, HALF // 2), bf16, kind="ExternalInput")
    # host-packed weights: 3 single sync DMAs instead of 11 gpsimd ones
    # (SWDGE descriptor generation serializes ~1us each on Pool and lands
    # weights after the PE needs them)
    wqwk_d = nc.dram_tensor("wqwk", (4 * P, C), f32r, kind="ExternalInput")
    vpk_d = nc.dram_tensor("vpk", (2 * P, XT_COLS_D), bf16,
                           kind="ExternalInput")
    bpk_d = nc.dram_tensor("bpk", (1, 2 * C), f32r, kind="ExternalInput")
    out_d = nc.dram_tensor("out", (C, HALF), bf16, kind="ExternalOutput")

    xt_ap = xt_d.ap()
    out_ap = out_d.ap()

    NT = N // P          # 128 n-tiles for the Gram phase
    CH_T = 16            # n-tiles per SBUF tile
    NCHUNK = NT // CH_T  # 8 chunks
    Exp = mybir.ActivationFunctionType.Exp
    Copy = mybir.ActivationFunctionType.Copy
    Ident = mybir.ActivationFunctionType.Identity

    with tile.TileContext(nc) as tc:
        with (
            tc.tile_pool(name="singles", bufs=1) as singles,
            tc.tile_pool(name="work", bufs=2) as work,
        ):
            # ---------- constants ----------
            identity = singles.tile([P, P], f32, tag="ident", name="ident")
            make_identity(nc, identity)
            identity_r = singles.tile([P, P], bf16, tag="identr", name="identr")
            nc.vector.tensor_copy(out=identity_r, in_=identity)
            identity_fr = singles.tile([P, P], f32r, tag="identfr", name="identfr")
            nc.vector.tensor_copy(out=identity_fr, in_=identity)
            # warm the ACT Exp table early so phase B's exp doesn't pay the
            # ~1.3us table load on the critical path
            warm = singles.tile([1, 2], f32, tag="warm", name="warm")
            nc.vector.memset(warm, 0.0)
            nc.scalar.activation(out=warm, in_=warm, func=Exp,
                                 bias=0.0, scale=1.0)

            # weights natural layout, f32r so every phase-B matmul runs at
            # full PE rate; DMAs for these are issued inside the chunk loop
            # (after the first data chunk) to keep the data stream in front
            wqwk_sb = singles.tile([P, 4, C], f32r, tag="wqwk", name="wqwk")
            vpk_sb = singles.tile([P, 2, XT_COLS_D], bf16, tag="vpk",
                                  name="vpk")
            bpk_sb = singles.tile([1, 2 * C], f32r, tag="bpk", name="bpk")
            wq_sb = [wqwk_sb[:, j, :] for j in range(2)]
            wk_sb = [wqwk_sb[:, 2 + j, :] for j in range(2)]
            wv_sb = [vpk_sb[:, j, 0:C] for j in range(2)]
            bv_col = [vpk_sb[:, j, C:C + 1] for j in range(2)]
            bq_row = bpk_sb[:, 0:C]
            bk_row = bpk_sb[:, C:2 * C]
            bkN_row = singles.tile([1, C], f32r, tag="bknr", name="bknr")

            # transposed weights wqT[e][:, c] = wq[c, e], wkT likewise
            wqT_sb = [singles.tile([P, C], f32r, tag=f"wqT{j}", name=f"wqT{j}") for j in range(2)]
            wkT_sb = [singles.tile([P, C], f32r, tag=f"wkT{j}", name=f"wkT{j}") for j in range(2)]

            gsb = [singles.tile([P, C + 2], f32r, tag=f"gsb{m}", name=f"gsb{m}") for m in range(2)]
            t1sb = [singles.tile([P, C], f32r, tag=f"t1sb{m}", name=f"t1sb{m}") for m in range(2)]
            attT_sb = [singles.tile([P, C], bf16, tag=f"attT{m}", name=f"attT{m}") for m in range(2)]
            mt_sb = [singles.tile([P, C], bf16, tag=f"mt{m}", name=f"mt{m}") for m in range(2)]
            r_sb = [singles.tile([P, 1], f32, tag=f"r{m}", name=f"r{m}") for m in range(2)]
            sq_row = singles.tile([1, C], f32r, tag="sqr", name="sqr")
            sk_row = singles.tile([1, C], f32r, tag="skr", name="skr")
            ssum_sb = [singles.tile([P, 1], f32, tag=f"ssum{m}", name=f"ssum{m}") for m in range(2)]
            rs_sb = [singles.tile([P, 1], f32, tag=f"rs{m}", name=f"rs{m}") for m in range(2)]

            # ---------- phase A: Gram accumulation ----------
            # Symmetric Gram: row-block 0 streams all 258 cols; row-block 1
            # streams only cols 128:258 (G10 = G01^T is reconstructed by one
            # PE transpose afterwards). Own-half chunks are each PE-transposed
            # back to natural layout (xn) right after their DMA lands.
            NRES = NCHUNK // 2
            CW = CH_T * P     # chunk width in spatial cols (2048)
            TB = 4            # transposes batched per PSUM bank
            CH_T_S = 8        # n-tiles per streamed (non-resident) SBUF tile
            M1_LO, M1_W = P, XT_COLS_D - P   # cols 128:258 (130 wide)
            NTRES = NRES // 2  # chunks transposed on-chip (rest DMA'd via xn_d)
            xn = [singles.tile([P, HALF], bf16, tag=f"xn{m}",
                               name=f"xn{m}") for m in range(2)]
            with (
                tc.tile_pool(name="psg", bufs=1, space=MemorySpace.PSUM) as psg,
                tc.tile_pool(name="psct", bufs=5, space=MemorySpace.PSUM) as psct,
                tc.tile_pool(name="xtrp", bufs=3) as xtrp,
                tc.tile_pool(name="xtp", bufs=5) as xtp,
            ):
                g_ps0 = psg.tile([P, C + 2], f32, tag="g0", name="g0")
                g_ps1 = psg.tile([P, M1_W], f32, tag="g1", name="g1")
                # interleave own/streamed chunks so PE is never starved and
                # the DMA queue never runs dry
                jobs = [("own", 0), ("str", 0), ("own", 1), ("str", 1),
                        ("str", 2), ("own", 2), ("str", 3), ("str", 4),
                        ("own", 3), ("str", 5), ("str", 6), ("str", 7)]
                wjobs = [(wsrc, wdst, jj, ii)
                         for (wsrc, wdst) in ((wq_sb, wqT_sb), (wk_sb, wkT_sb))
                         for jj in range(2) for ii in range(2)]
                nt = 0
                ncopy = 0
                for ji, (kind, ch) in enumerate(jobs):
                    if kind == "own":
                        ctiles = CH_T
                        xt = xtrp.tile([P, CH_T, XT_COLS_D], bf16, tag="xtr",
                                       name="xtr")
                        if ch == 0:
                            # split the first chunk so PE starts ~4x sooner
                            QT = CH_T // 4
                            for q in range(4):
                                nc.sync.dma_start(
                                    out=xt[:, q * QT:(q + 1) * QT, :],
                                    in_=xt_ap[q * QT * P:(q + 1) * QT * P, :]
                                        .rearrange("(t p) c -> p t c", p=P),
                                )
                        else:
                            nc.sync.dma_start(
                                out=xt,
                                in_=xt_ap[ch * CH_T * P:(ch + 1) * CH_T * P, :]
                                    .rearrange("(t p) c -> p t c", p=P),
                            )
                    else:
                        ctiles = CH_T_S
                        xt = xtp.tile([P, CH_T_S, XT_COLS_D], bf16, tag="xt",
                                      name="xt")
                        row0 = NRES * CH_T * P + ch * CH_T_S * P
                        nc.sync.dma_start(
                            out=xt,
                            in_=xt_ap[row0:row0 + CH_T_S * P, :]
                                .rearrange("(t p) c -> p t c", p=P),
                        )
                    for t in range(ctiles):
                        nc.tensor.matmul(
                            g_ps0[:, 0:C + 2],
                            xt[:, t, 0:P],
                            xt[:, t, 0:C + 2],
                            start=(nt == 0), stop=(nt == NT - 1),
                        )
                        nc.tensor.matmul(
                            g_ps1[:, 0:M1_W],
                            xt[:, t, P:C],
                            xt[:, t, M1_LO:XT_COLS_D],
                            start=(nt == 0), stop=(nt == NT - 1),
                        )
                        nt += 1
                    if kind == "own" and ch < NTRES:
                        # transpose to natural layout; PSUM->SBUF copies
                        # alternate ACT/DVE
                        for m in range(2):
                            for tb in range(CH_T // TB):
                                tps = psct.tile([P, TB * P], bf16, tag="tps",
                                                name="tps")
                                for k in range(TB):
                                    t = tb * TB + k
                                    nc.tensor.transpose(
                                        tps[:, k * P:(k + 1) * P],
                                        xt[:, t, m * P:(m + 1) * P],
                                        identity_r)
                                dst = xn[m][:, ch * CW + tb * TB * P:
                                            ch * CW + (tb + 1) * TB * P]
                                if ncopy % 2 == 0:
                                    nc.vector.tensor_copy(out=dst, in_=tps)
                                else:
                                    nc.scalar.activation(out=dst, in_=tps,
                                                         func=Copy,
                                                         bias=0.0, scale=1.0)
                                ncopy += 1
                    elif kind == "str" and ch < len(wjobs):
                        wsrc, wdst, jj, ii = wjobs[ch]
                        ps = psct.tile([P, P], f32r, tag="wt", name="wt",
                                       bufs=1)
                        nc.tensor.transpose(
                            ps, wsrc[ii][:, jj * P:(jj + 1) * P], identity_fr)
                        nc.vector.tensor_copy(
                            out=wdst[jj][:, ii * P:(ii + 1) * P], in_=ps)
                    if ji == 0:
                        # packed weights ride the same HWDGE queue right
                        # behind the first data chunk
                        nc.sync.dma_start(
                            out=wqwk_sb,
                            in_=wqwk_d.ap().rearrange("(j p) c -> p j c", p=P))
                        nc.sync.dma_start(
                            out=vpk_sb,
                            in_=vpk_d.ap().rearrange("(j p) c -> p j c", p=P))
                        nc.sync.dma_start(out=bpk_sb, in_=bpk_d.ap())
                        nc.vector.tensor_scalar_mul(bkN_row, bk_row, float(N))

                # natural-layout chunks 2-3 arrive by DMA in the gap after
                # the Gram input finishes (queued behind all xt loads)
                for m in range(2):
                    nc.sync.dma_start(
                        out=xn[m][:, NTRES * CW:HALF],
                        in_=xn_d.ap()[m * P:(m + 1) * P, :])

                # G to SBUF: block row 0 fully (split across DVE/ACT; the
                # 128:258 slice lands first so the fixup transpose can start),
                # block row 1 cols 128:258 from PSUM via ACT
                nc.vector.tensor_copy(out=gsb[0][:, P:XT_COLS_D],
                                      in_=g_ps0[:, P:XT_COLS_D])
                nc.scalar.activation(out=gsb[0][:, 0:P], in_=g_ps0[:, 0:P],
                                     func=Copy, bias=0.0, scale=1.0)
                nc.scalar.activation(out=gsb[1][:, M1_LO:XT_COLS_D],
                                     in_=g_ps1, func=Copy, bias=0.0, scale=1.0)

            # ---------- phase B: energy^T, exp, M, r ----------
            # Everything is computed directly in the TRANSPOSED (d, c)
            # orientation (G is symmetric), so no PE<->DVE transpose
            # ping-pong. exp is taken without max-subtraction (energies
            # here are |e|/sqrt(N) < ~50, exp < 1e20, far from fp32
            # overflow); the 1/rowsum normalization is folded into the
            # phase-C output scale.
            with tc.tile_pool(name="psb", bufs=1, space=MemorySpace.PSUM) as psb:
                # G fixup: G10 = (G01)^T via one PE transpose
                tfix = psb.tile([P, P], f32r, tag="wt", name="tfix", bufs=1)
                nc.tensor.transpose(tfix, gsb[0][:, P:C], identity_fr)
                nc.vector.tensor_copy(out=gsb[1][:, 0:P], in_=tfix)

                # sq' = wq sx (row), sk' = wk sx (row) — early, they feed the
                # rank-1 energy terms; copies split ACT/DVE
                sq_ps = psb.tile([1, C], f32, tag="rps", name="sqp", bufs=2)
                for e in range(2):
                    nc.tensor.matmul(
                        sq_ps, gsb[e][:, C:C + 1], wqT_sb[e][:, 0:C],
                        start=(e == 0), stop=(e == 1))
                nc.scalar.activation(out=sq_row, in_=sq_ps, func=Copy,
                                     bias=0.0, scale=1.0)
                sk_ps = psb.tile([1, C], f32, tag="rps", name="skp", bufs=2)
                for e in range(2):
                    nc.tensor.matmul(
                        sk_ps, gsb[e][:, C:C + 1], wkT_sb[e][:, 0:C],
                        start=(e == 0), stop=(e == 1))
                nc.vector.tensor_copy(out=sk_row, in_=sk_ps)

                # T1q[a, c] = (G wq^T)[a, c]; e=0 contributions first so they
                # run while the G fixup completes
                t1_ps = [psb.tile([P, C], f32, tag="tmp", name=f"t1p{a}",
                                  bufs=3) for a in range(2)]
                for a in range(2):
                    nc.tensor.matmul(
                        t1_ps[a], gsb[0][:, a * P:(a + 1) * P],
                        wqT_sb[0][:, 0:C], start=True, stop=False)
                for a in range(2):
                    nc.tensor.matmul(
                        t1_ps[a], gsb[1][:, a * P:(a + 1) * P],
                        wqT_sb[1][:, 0:C], start=False, stop=True)
                nc.vector.tensor_copy(out=t1sb[0], in_=t1_ps[0])
                nc.scalar.activation(out=t1sb[1], in_=t1_ps[1], func=Copy,
                                     bias=0.0, scale=1.0)

                # energyT (raw, unscaled) per d-tile:
                # energyT[d, c] = (wk G wq^T)[d, c] + bk[d] sq'[c]
                #                 + sk'[d] bq[c] + N bk[d] bq[c]
                eT_ps = [psb.tile([P, C], f32, tag=f"eps{dt}", name=f"eps{dt}")
                         for dt in range(2)]
                for dt in range(2):
                    ds_ = (dt * P, (dt + 1) * P)
                    nc.tensor.matmul(
                        eT_ps[dt],
                        wkT_sb[0][:, dt * P:(dt + 1) * P],
                        t1sb[0][:, 0:C],
                        start=True, stop=False,
                    )
                    nc.tensor.matmul(eT_ps[dt], bkN_row[:, ds_[0]:ds_[1]],
                                     bq_row[:, 0:C], start=False, stop=False)
                    nc.tensor.matmul(eT_ps[dt], bk_row[:, ds_[0]:ds_[1]],
                                     sq_row[:, 0:C], start=False, stop=False)
                    nc.tensor.matmul(
                        eT_ps[dt],
                        wkT_sb[1][:, dt * P:(dt + 1) * P],
                        t1sb[1][:, 0:C],
                        start=False, stop=False,
                    )
                    nc.tensor.matmul(eT_ps[dt], sk_row[:, ds_[0]:ds_[1]],
                                     bq_row[:, 0:C], start=False, stop=True)
                    # attT (unnormalized): exp(energyT / sqrt(N))
                    nc.scalar.activation(
                        out=attT_sb[dt], in_=eT_ps[dt], func=Exp,
                        bias=0.0, scale=1.0 / SQRT_N)

                ones_col = singles.tile([P, 1], bf16, tag="ones", name="ones")
                nc.vector.memset(ones_col, 1.0)

                # row sums: ssum[c] = sum_d attT[d, c] (column via matmul)
                dg_bf = [work.tile([P, P], bf16, tag=f"dg{e}", name=f"dg{e}")
                         for e in range(2)]
                for ct in range(2):
                    ps = psb.tile([P, 1], f32, tag="rps", name="rps", bufs=2)
                    for d in range(2):
                        nc.tensor.matmul(
                            ps, attT_sb[d][:, ct * P:(ct + 1) * P], ones_col,
                            start=(d == 0), stop=(d == 1))
                    nc.vector.tensor_copy(out=ssum_sb[ct], in_=ps)
                    nc.vector.reciprocal(out=rs_sb[ct], in_=ssum_sb[ct])
                    # diag(ssum) as bf16 for the in-PSUM diagonal matmul
                    nc.vector.tensor_scalar_mul(dg_bf[ct], identity_r,
                                                ssum_sb[ct])

                # MT[e][:, c] = M~[c, e] = sum_d att~[c, d] wv[d, e]
                # (+ diag(ssum) added by a PE matmul so phase C's rs scale
                # yields M x + x); copies split DVE/ACT
                for e in range(2):
                    ps = psb.tile([P, C], f32, tag="tmp", name=f"mtp{e}",
                                  bufs=3)
                    for d in range(2):
                        nc.tensor.matmul(
                            ps,
                            wv_sb[d][:, e * P:(e + 1) * P],
                            attT_sb[d][:, 0:C],
                            start=(d == 0), stop=False,
                        )
                    nc.tensor.matmul(
                        ps[:, e * P:(e + 1) * P], dg_bf[e], identity_r,
                        start=False, stop=True)
                    if e == 0:
                        nc.vector.tensor_copy(out=mt_sb[e], in_=ps)
                    else:
                        nc.scalar.activation(out=mt_sb[e], in_=ps, func=Copy,
                                             bias=0.0, scale=1.0)

                # r[c] = rs[c] * sum_d att~[c, d] bv[d]
                for ct in range(2):
                    ps = psb.tile([P, 1], f32, tag="rps", name="rps", bufs=2)
                    for d in range(2):
                        nc.tensor.matmul(
                            ps, attT_sb[d][:, ct * P:(ct + 1) * P], bv_col[d],
                            start=(d == 0), stop=(d == 1))
                    nc.vector.tensor_copy(out=r_sb[ct], in_=ps)
                    nc.vector.tensor_mul(r_sb[ct], r_sb[ct], rs_sb[ct])

            # ---------- phase C: out = (M + I) x + r ----------
            MMW = 512         # matmul free width (one PSUM bank of f32)
            mult = mybir.AluOpType.mult
            add = mybir.AluOpType.add
            with (
                tc.tile_pool(name="psc", bufs=2, space=MemorySpace.PSUM) as psc,
                tc.tile_pool(name="outp", bufs=3) as outp,
            ):
                for ch in range(NRES):
                    for ct in range(2):
                        ot = outp.tile([P, CW], bf16, tag="ot", name="ot")
                        ps = psc.tile([P, CW], f32, tag="ops", name="ops")
                        for s in range(CW // MMW):
                            sl = slice(s * MMW, (s + 1) * MMW)
                            for e in range(2):
                                nc.tensor.matmul(
                                    ps[:, sl],
                                    mt_sb[e][:, ct * P:(ct + 1) * P],
                                    xn[e][:, ch * CW + s * MMW:
                                           ch * CW + (s + 1) * MMW],
                                    start=(e == 0), stop=(e == 1),
                                )
                        # ot = rs*psum + r (per-partition scale+bias), split
                        # ACT || DVE; the final tiles use finer slices so the
                        # last DMA starts (and ends) sooner
                        nslc = 4 if ch == NRES - 1 else 2
                        sw = CW // nslc
                        for s in range(nslc):
                            sl = slice(s * sw, (s + 1) * sw)
                            if s % 2 == 0:
                                nc.scalar.activation(
                                    out=ot[:, sl], in_=ps[:, sl], func=Ident,
                                    bias=r_sb[ct], scale=rs_sb[ct])
                            else:
                                nc.vector.tensor_scalar(
                                    out=ot[:, sl], in0=ps[:, sl],
                                    scalar1=rs_sb[ct], scalar2=r_sb[ct],
                                    op0=mult, op1=add)
                            nc.sync.dma_start(
                                out=out_ap[ct * P:(ct + 1) * P,
                                           ch * CW + s * sw:
                                           ch * CW + (s + 1) * sw],
                                in_=ot[:, sl])

    nc.compile()
    return nc


def _get_nc():
    key = "v2"
    if key not in _BUILD_CACHE:
        _BUILD_CACHE[key] = _build()
    return _BUILD_CACHE[key]


def kernel(x, wq, bq, wk, bk, wv, bv):
    global LAST_RESULT
    from concourse.bass_utils import run_bass_kernel_spmd

    nc = _get_nc()

    x = np.ascontiguousarray(np.asarray(x, dtype=np.float32))
    xf = x.reshape(B, C, N)
    wq = np.ascontiguousarray(np.asarray(wq, dtype=np.float32))
    wk = np.ascontiguousarray(np.asarray(wk, dtype=np.float32))
    wv = np.ascontiguousarray(np.asarray(wv, dtype=np.float32))
    bq = np.ascontiguousarray(np.asarray(bq, dtype=np.float32))
    bk = np.ascontiguousarray(np.asarray(bk, dtype=np.float32))
    bv = np.ascontiguousarray(np.asarray(bv, dtype=np.float32))

    in_maps = _make_in_maps(xf, wq, bq, wk, bk, wv, bv)

    res = run_bass_kernel_spmd(nc, in_maps, core_ids=list(range(NCORES)))
    LAST_RESULT = res

    out = np.empty((B, C, N), dtype=np.float32)
    for i in range(NCORES):
        b, h = i // 2, i % 2
        out[b, :, h * HALF:(h + 1) * HALF] = np.asarray(
            res.results[i]["out"]).astype(np.float32)
    return out.reshape(B, C, W, H)


# ---------------------------------------------------------------------------
# Dev-loop helpers (not used by the grading path)
# ---------------------------------------------------------------------------

def timeline_ns():
    """Cost-model simulated duration of one core's program (ns)."""
    from concourse.timeline_sim import TimelineSim
    nc = _get_nc()
    ts = TimelineSim(nc)
    return ts.simulate()


def _make_in_maps(xf, wq, bq, wk, bk, wv, bv):
    ones_pad = np.zeros((N, 2), dtype=np.float32)
    ones_pad[:, 0] = 1.0
    wqwk = np.ascontiguousarray(np.concatenate([wq, wk], axis=0))
    vpk = np.concatenate(
        [wv, bv[:, None], np.zeros((C, 1), np.float32)],
        axis=1).astype(ml_dtypes.bfloat16)
    bpk = np.concatenate([bq, bk])[None, :].astype(np.float32)
    in_maps = []
    for i in range(NCORES):
        b, h = i // 2, i % 2
        xTb = np.concatenate([xf[b].T, ones_pad], axis=1)
        # own spatial half first: the kernel keeps the first NCHUNK/2 chunks
        # resident and derives its output columns from them
        xt = np.concatenate([xTb[h * HALF:(h + 1) * HALF],
                             xTb[(1 - h) * HALF:(2 - h) * HALF]],
                            axis=0).astype(ml_dtypes.bfloat16)
        xn = np.ascontiguousarray(
            xf[b][:, h * HALF + HALF // 2:(h + 1) * HALF]
        ).astype(ml_dtypes.bfloat16)
        in_maps.append({
            "xt": xt, "xn": xn,
            "wqwk": wqwk, "vpk": vpk, "bpk": bpk,
        })
    return in_maps


# revision 36
# speedup vs baseline: 1.2365x; 1.0820x over previous
"""Channel cross-attention kernel for Trainium2 (8 NeuronCores).

Math (exact restructuring of the reference):
    xf = x.reshape(B, C, N)
    q = wq xf + bq;  k = wk xf + bk;  v = wv xf + bv
    energy = q k^T = wq G wk^T + (wq sx) bk^T + bq (wk sx)^T + N bq bk^T
        where G = xf xf^T (C x C Gram), sx = xf @ 1 (row sums)
    att = softmax(energy / sqrt(N))
    out = att v + xf = (att wv) xf + (att bv) 1^T + xf = M xf + r 1^T + xf

Sharding: 8 cores, core i handles sample b=i//2, spatial half h=i%2.
Each core computes G over the FULL sample (redundantly within the pair, no
cross-core communication) and produces its own spatial half of the output.

Precision: x streams in bf16 (halves HBM read), out streams back bf16
(halves HBM write; host upcasts), everything between Gram and the output
matmul runs in f32(r). The Gram uses the symmetry G = G^T: the second
row-block only streams cols 128:258 and the missing 128x128 block is
reconstructed by one PE transpose.

Host prep per core: xt = xf[b].T (N, C+2: data, ones, pad) in bf16, rows
ordered own-spatial-half first. The Gram phase streams all rows; the own
half stays resident in SBUF and is transposed back on-chip for the output
phase.
"""

import os
import sys

for _p in ("/opt/trn_rl_repo", "/root/.axon_site/_ro/trn_rl_repo"):
    if os.path.isdir(_p) and _p not in sys.path:
        sys.path.append(_p)

import numpy as np
import ml_dtypes

# ---- problem constants (hardcoded; must match setup_inputs) ----
B, C, W, H = 4, 256, 128, 128
N = W * H            # 16384
HALF = N // 2        # 8192
P = 128              # partitions
NCORES = 8
SQRT_N = float(np.sqrt(N))   # 128.0
XT_COLS_D = C + 2            # xt DRAM row: 256 data cols, ones col, zero pad

GRAM_DT = "bf16"     # informational (printed by test harness)
MM_DT = "bf16"

_BUILD_CACHE = {}
LAST_RESULT = None   # BassKernelResults of the most recent run (for test harness)


def _build():
    import concourse.bacc as bacc
    import concourse.mybir as mybir
    import concourse.tile as tile
    from concourse.bass import MemorySpace
    from concourse.masks import make_identity

    f32 = mybir.dt.float32
    f32r = mybir.dt.float32r
    bf16 = mybir.dt.bfloat16

    nc = bacc.Bacc("TRN2", target_bir_lowering=False)

    xt_d = nc.dram_tensor("xt", (N, XT_COLS_D), bf16, kind="ExternalInput")
    # natural-layout own-half cols 4096:8192 (phase C chunks 2-3): loaded
    # directly instead of PE-transposing, filling the post-input DMA idle gap
    xn_d = nc.dram_tensor("xn", (C, HALF // 2), bf16, kind="ExternalInput")
    # host-packed weights: 3 single sync DMAs instead of 11 gpsimd ones
    # (SWDGE descriptor generation serializes ~1us each on Pool and lands
    # weights after the PE needs them)
    wqwk_d = nc.dram_tensor("wqwk", (4 * P, C), f32r, kind="ExternalInput")
    vpk_d = nc.dram_tensor("vpk", (2 * P, XT_COLS_D), bf16,
                           kind="ExternalInput")
    bpk_d = nc.dram_tensor("bpk", (1, 2 * C), f32r, kind="ExternalInput")
    out_d = nc.dram_tensor("out", (C, HALF), bf16, kind="ExternalOutput")

    xt_ap = xt_d.ap()
    out_ap = out_d.ap()

    NT = N // P          # 128 n-tiles for the Gram phase
    CH_T = 16            # n-tiles per SBUF tile
    NCHUNK = NT // CH_T  # 8 chunks
    Exp = mybir.ActivationFunctionType.Exp
    Copy = mybir.ActivationFunctionType.Copy
    Ident = mybir.ActivationFunctionType.Identity

    with tile.TileContext(nc) as tc:
        with (
            tc.tile_pool(name="singles", bufs=1) as singles,
            tc.tile_pool(name="work", bufs=2) as work,
        ):
            # ---------- constants ----------
            identity = singles.tile([P, P], f32, tag="ident", name="ident")
            make_identity(nc, identity)
            identity_r = singles.tile([P, P], bf16, tag="identr", name="identr")
            nc.vector.tensor_copy(out=identity_r, in_=identity)
            identity_fr = singles.tile([P, P], f32r, tag="identfr", name="identfr")
            nc.vector.tensor_copy(out=identity_fr, in_=identity)
            # warm the ACT Exp table early so phase B's exp doesn't pay the
            # ~1.3us table load on the critical path
            warm = singles.tile([1, 2], f32, tag="warm", name="warm")
            nc.vector.memset(warm, 0.0)
            nc.scalar.activation(out=warm, in_=warm, func=Exp,
                                 bias=0.0, scale=1.0)

            # weights natural layout, f32r so every phase-B matmul runs at
            # full PE rate; DMAs for these are issued inside the chunk loop
            # (after the first data chunk) to keep the data stream in front
            wqwk_sb = singles.tile([P, 4, C], f32r, tag="wqwk", name="wqwk")
            vpk_sb = singles.tile([P, 2, XT_COLS_D], bf16, tag="vpk",
                                  name="vpk")
            bpk_sb = singles.tile([1, 2 * C], f32r, tag="bpk", name="bpk")
            wq_sb = [wqwk_sb[:, j, :] for j in range(2)]
            wk_sb = [wqwk_sb[:, 2 + j, :] for j in range(2)]
            wv_sb = [vpk_sb[:, j, 0:C] for j in range(2)]
            bv_col = [vpk_sb[:, j, C:C + 1] for j in range(2)]
            bq_row = bpk_sb[:, 0:C]
            bk_row = bpk_sb[:, C:2 * C]
            bkN_row = singles.tile([1, C], f32r, tag="bknr", name="bknr")

            # transposed weights wqT[e][:, c] = wq[c, e], wkT likewise
            wqT_sb = [singles.tile([P, C], f32r, tag=f"wqT{j}", name=f"wqT{j}") for j in range(2)]
            wkT_sb = [singles.tile([P, C], f32r, tag=f"wkT{j}", name=f"wkT{j}") for j in range(2)]

            gsb = [singles.tile([P, C + 2], f32r, tag=f"gsb{m}", name=f"gsb{m}") for m in range(2)]
            t1sb = [singles.tile([P, C], f32r, tag=f"t1sb{m}", name=f"t1sb{m}") for m in range(2)]
            attT_sb = [singles.tile([P, C], bf16, tag=f"attT{m}", name=f"attT{m}") for m in range(2)]
            mt_sb = [singles.tile([P, C], bf16, tag=f"mt{m}", name=f"mt{m}") for m in range(2)]
            r_sb = [singles.tile([P, 1], f32, tag=f"r{m}", name=f"r{m}") for m in range(2)]
            sq_row = singles.tile([1, C], f32r, tag="sqr", name="sqr")
            sk_row = singles.tile([1, C], f32r, tag="skr", name="skr")
            ssum_sb = [singles.tile([P, 1], f32, tag=f"ssum{m}", name=f"ssum{m}") for m in range(2)]
            rs_sb = [singles.tile([P, 1], f32, tag=f"rs{m}", name=f"rs{m}") for m in range(2)]

            # ---------- phase A: Gram accumulation ----------
            # Symmetric Gram: row-block 0 streams all 258 cols; row-block 1
            # streams only cols 128:258 (G10 = G01^T is reconstructed by one
            # PE transpose afterwards). Own-half chunks are each PE-transposed
            # back to natural layout (xn) right after their DMA lands.
            NRES = NCHUNK // 2
            CW = CH_T * P     # chunk width in spatial cols (2048)
            TB = 4            # transposes batched per PSUM bank
            CH_T_S = 8        # n-tiles per streamed (non-resident) SBUF tile
            M1_LO, M1_W = P, XT_COLS_D - P   # cols 128:258 (130 wide)
            NTRES = NRES // 2  # chunks transposed on-chip (rest DMA'd via xn_d)
            xn = [singles.tile([P, HALF], bf16, tag=f"xn{m}",
                               name=f"xn{m}") for m in range(2)]
            with (
                tc.tile_pool(name="psg", bufs=1, space=MemorySpace.PSUM) as psg,
                tc.tile_pool(name="psct", bufs=5, space=MemorySpace.PSUM) as psct,
                tc.tile_pool(name="xtrp", bufs=3) as xtrp,
                tc.tile_pool(name="xtp", bufs=5) as xtp,
            ):
                g_ps0 = psg.tile([P, C + 2], f32, tag="g0", name="g0")
                g_ps1 = psg.tile([P, M1_W], f32, tag="g1", name="g1")
                # interleave own/streamed chunks so PE is never starved and
                # the DMA queue never runs dry
                jobs = [("own", 0), ("str", 0), ("own", 1), ("str", 1),
                        ("str", 2), ("own", 2), ("str", 3), ("str", 4),
                        ("own", 3), ("str", 5), ("str", 6), ("str", 7)]
                wjobs = [(wsrc, wdst, jj, ii)
                         for (wsrc, wdst) in ((wq_sb, wqT_sb), (wk_sb, wkT_sb))
                         for jj in range(2) for ii in range(2)]
                nt = 0
                ncopy = 0
                for ji, (kind, ch) in enumerate(jobs):
                    if kind == "own":
                        ctiles = CH_T
                        xt = xtrp.tile([P, CH_T, XT_COLS_D], bf16, tag="xtr",
                                       name="xtr")
                        if ch == 0:
                            # split the first chunk so PE starts ~4x sooner
                            QT = CH_T // 4
                            for q in range(4):
                                nc.sync.dma_start(
                                    out=xt[:, q * QT:(q + 1) * QT, :],
                                    in_=xt_ap[q * QT * P:(q + 1) * QT * P, :]
                                        .rearrange("(t p) c -> p t c", p=P),
                                )
                        else:
                            nc.sync.dma_start(
                                out=xt,
                                in_=xt_ap[ch * CH_T * P:(ch + 1) * CH_T * P, :]
                                    .rearrange("(t p) c -> p t c", p=P),
                            )
                    else:
                        ctiles = CH_T_S
                        xt = xtp.tile([P, CH_T_S, XT_COLS_D], bf16, tag="xt",
                                      name="xt")
                        row0 = NRES * CH_T * P + ch * CH_T_S * P
                        nc.sync.dma_start(
                            out=xt,
                            in_=xt_ap[row0:row0 + CH_T_S * P, :]
                                .rearrange("(t p) c -> p t c", p=P),
                        )
                    for t in range(ctiles):
                        nc.tensor.matmul(
                            g_ps0[:, 0:C + 2],
                            xt[:, t, 0:P],
                            xt[:, t, 0:C + 2],
                            start=(nt == 0), stop=(nt == NT - 1),
                        )
                        nc.tensor.matmul(
                            g_ps1[:, 0:M1_W],
                            xt[:, t, P:C],
                            xt[:, t, M1_LO:XT_COLS_D],
                            start=(nt == 0), stop=(nt == NT - 1),
                        )
                        nt += 1
                    if kind == "own" and ch < NTRES:
                        # transpose to natural layout; PSUM->SBUF copies
                        # alternate ACT/DVE
                        for m in range(2):
                            for tb in range(CH_T // TB):
                                tps = psct.tile([P, TB * P], bf16, tag="tps",
                                                name="tps")
                                for k in range(TB):
                                    t = tb * TB + k
                                    nc.tensor.transpose(
                                        tps[:, k * P:(k + 1) * P],
                                        xt[:, t, m * P:(m + 1) * P],
                                        identity_r)
                                dst = xn[m][:, ch * CW + tb * TB * P:
                                            ch * CW + (tb + 1) * TB * P]
                                if ncopy % 2 == 0:
                                    nc.vector.tensor_copy(out=dst, in_=tps)
                                else:
                                    nc.scalar.activation(out=dst, in_=tps,
                                                         func=Copy,
                                                         bias=0.0, scale=1.0)
                                ncopy += 1
                    elif kind == "str" and ch < len(wjobs):
                        wsrc, wdst, jj, ii = wjobs[ch]
                        ps = psct.tile([P, P], f32r, tag="wt", name="wt",
                                       bufs=1)
                        nc.tensor.transpose(
                            ps, wsrc[ii][:, jj * P:(jj + 1) * P], identity_fr)
                        nc.vector.tensor_copy(
                            out=wdst[jj][:, ii * P:(ii + 1) * P], in_=ps)
                    if ji == 0:
                        # packed weights ride the same HWDGE queue right
                        # behind the first data chunk
                        nc.sync.dma_start(
                            out=wqwk_sb,
                            in_=wqwk_d.ap().rearrange("(j p) c -> p j c", p=P))
                        nc.sync.dma_start(
                            out=vpk_sb,
                            in_=vpk_d.ap().rearrange("(j p) c -> p j c", p=P))
                        nc.sync.dma_start(out=bpk_sb, in_=bpk_d.ap())
                        nc.vector.tensor_scalar_mul(bkN_row, bk_row, float(N))

                # natural-layout chunks 2-3 arrive by DMA in the gap after
                # the Gram input finishes (queued behind all xt loads)
                for m in range(2):
                    nc.sync.dma_start(
                        out=xn[m][:, NTRES * CW:HALF],
                        in_=xn_d.ap()[m * P:(m + 1) * P, :])

                # G to SBUF: block row 0 fully (split across DVE/ACT; the
                # 128:258 slice lands first so the fixup transpose can start),
                # block row 1 cols 128:258 from PSUM via ACT
                nc.vector.tensor_copy(out=gsb[0][:, P:XT_COLS_D],
                                      in_=g_ps0[:, P:XT_COLS_D])
                nc.scalar.activation(out=gsb[0][:, 0:P], in_=g_ps0[:, 0:P],
                                     func=Copy, bias=0.0, scale=1.0)
                nc.scalar.activation(out=gsb[1][:, M1_LO:XT_COLS_D],
                                     in_=g_ps1, func=Copy, bias=0.0, scale=1.0)

            # ---------- phase B: energy^T, exp, M, r ----------
            # Everything is computed directly in the TRANSPOSED (d, c)
            # orientation (G is symmetric), so no PE<->DVE transpose
            # ping-pong. exp is taken without max-subtraction (energies
            # here are |e|/sqrt(N) < ~50, exp < 1e20, far from fp32
            # overflow); the 1/rowsum normalization is folded into the
            # phase-C output scale.
            with tc.tile_pool(name="psb", bufs=1, space=MemorySpace.PSUM) as psb:
                # G fixup: G10 = (G01)^T via one PE transpose
                tfix = psb.tile([P, P], f32r, tag="wt", name="tfix", bufs=1)
                nc.tensor.transpose(tfix, gsb[0][:, P:C], identity_fr)
                nc.vector.tensor_copy(out=gsb[1][:, 0:P], in_=tfix)

                # sq' = wq sx (row), sk' = wk sx (row) — early, they feed the
                # rank-1 energy terms; copies split ACT/DVE
                sq_ps = psb.tile([1, C], f32, tag="rps", name="sqp", bufs=2)
                for e in range(2):
                    nc.tensor.matmul(
                        sq_ps, gsb[e][:, C:C + 1], wqT_sb[e][:, 0:C],
                        start=(e == 0), stop=(e == 1))
                nc.scalar.activation(out=sq_row, in_=sq_ps, func=Copy,
                                     bias=0.0, scale=1.0)
                sk_ps = psb.tile([1, C], f32, tag="rps", name="skp", bufs=2)
                for e in range(2):
                    nc.tensor.matmul(
                        sk_ps, gsb[e][:, C:C + 1], wkT_sb[e][:, 0:C],
                        start=(e == 0), stop=(e == 1))
                nc.vector.tensor_copy(out=sk_row, in_=sk_ps)

                # T1q[a, c] = (G wq^T)[a, c]; e=0 contributions first so they
                # run while the G fixup completes
                t1_ps = [psb.tile([P, C], f32, tag="tmp", name=f"t1p{a}",
                                  bufs=3) for a in range(2)]
                for a in range(2):
                    nc.tensor.matmul(
                        t1_ps[a], gsb[0][:, a * P:(a + 1) * P],
                        wqT_sb[0][:, 0:C], start=True, stop=False)
                for a in range(2):
                    nc.tensor.matmul(
                        t1_ps[a], gsb[1][:, a * P:(a + 1) * P],
                        wqT_sb[1][:, 0:C], start=False, stop=True)
                nc.vector.tensor_copy(out=t1sb[0], in_=t1_ps[0])
                nc.scalar.activation(out=t1sb[1], in_=t1_ps[1], func=Copy,
                                     bias=0.0, scale=1.0)

                # energyT (raw, unscaled) per d-tile:
                # energyT[d, c] = (wk G wq^T)[d, c] + bk[d] sq'[c]
                #                 + sk'[d] bq[c] + N bk[d] bq[c]
                eT_ps = [psb.tile([P, C], f32, tag=f"eps{dt}", name=f"eps{dt}")
                         for dt in range(2)]
                for dt in range(2):
                    ds_ = (dt * P, (dt + 1) * P)
                    nc.tensor.matmul(
                        eT_ps[dt],
                        wkT_sb[0][:, dt * P:(dt + 1) * P],
                        t1sb[0][:, 0:C],
                        start=True, stop=False,
                    )
                    nc.tensor.matmul(eT_ps[dt], bkN_row[:, ds_[0]:ds_[1]],
                                     bq_row[:, 0:C], start=False, stop=False)
                    nc.tensor.matmul(eT_ps[dt], bk_row[:, ds_[0]:ds_[1]],
                                     sq_row[:, 0:C], start=False, stop=False)
                    nc.tensor.matmul(
                        eT_ps[dt],
                        wkT_sb[1][:, dt * P:(dt + 1) * P],
                        t1sb[1][:, 0:C],
                        start=False, stop=False,
                    )
                    nc.tensor.matmul(eT_ps[dt], sk_row[:, ds_[0]:ds_[1]],
                                     bq_row[:, 0:C], start=False, stop=True)
                    # attT (unnormalized): exp(energyT / sqrt(N))
                    nc.scalar.activation(
                        out=attT_sb[dt], in_=eT_ps[dt], func=Exp,
                        bias=0.0, scale=1.0 / SQRT_N)

                ones_col = singles.tile([P, 1], bf16, tag="ones", name="ones")
                nc.vector.memset(ones_col, 1.0)

                # row sums: ssum[c] = sum_d attT[d, c] (column via matmul)
                dg_bf = [work.tile([P, P], bf16, tag=f"dg{e}", name=f"dg{e}")
                         for e in range(2)]
                for ct in range(2):
                    ps = psb.tile([P, 1], f32, tag="rps", name="rps", bufs=2)
                    for d in range(2):
                        nc.tensor.matmul(
                            ps, attT_sb[d][:, ct * P:(ct + 1) * P], ones_col,
                            start=(d == 0), stop=(d == 1))
                    nc.vector.tensor_copy(out=ssum_sb[ct], in_=ps)
                    nc.vector.reciprocal(out=rs_sb[ct], in_=ssum_sb[ct])
                    # diag(ssum) as bf16 for the in-PSUM diagonal matmul
                    nc.vector.tensor_scalar_mul(dg_bf[ct], identity_r,
                                                ssum_sb[ct])

                # MT[e][:, c] = M~[c, e] = sum_d att~[c, d] wv[d, e]
                # (+ diag(ssum) added by a PE matmul so phase C's rs scale
                # yields M x + x); copies split DVE/ACT
                for e in range(2):
                    ps = psb.tile([P, C], f32, tag="tmp", name=f"mtp{e}",
                                  bufs=3)
                    for d in range(2):
                        nc.tensor.matmul(
                            ps,
                            wv_sb[d][:, e * P:(e + 1) * P],
                            attT_sb[d][:, 0:C],
                            start=(d == 0), stop=False,
                        )
                    nc.tensor.matmul(
                        ps[:, e * P:(e + 1) * P], dg_bf[e], identity_r,
                        start=False, stop=True)
                    if e == 0:
                        nc.vector.tensor_copy(out=mt_sb[e], in_=ps)
                    else:
                        nc.scalar.activation(out=mt_sb[e], in_=ps, func=Copy,
                                             bias=0.0, scale=1.0)

                # r[c] = rs[c] * sum_d att~[c, d] bv[d]
                for ct in range(2):
                    ps = psb.tile([P, 1], f32, tag="rps", name="rps", bufs=2)
                    for d in range(2):
                        nc.tensor.matmul(
                            ps, attT_sb[d][:, ct * P:(ct + 1) * P], bv_col[d],
                            start=(d == 0), stop=(d == 1))
                    nc.vector.tensor_copy(out=r_sb[ct], in_=ps)
                    nc.vector.tensor_mul(r_sb[ct], r_sb[ct], rs_sb[ct])

            # ---------- phase C: out = (M + I) x + r ----------
            MMW = 512         # matmul free width (one PSUM bank of f32)
            mult = mybir.AluOpType.mult
            add = mybir.AluOpType.add
            with (
                tc.tile_pool(name="psc", bufs=2, space=MemorySpace.PSUM) as psc,
                tc.tile_pool(name="outp", bufs=3) as outp,
            ):
                hw_ = CW // 2
                for ch in range(NRES):
                    for ct in range(2):
                        ot = outp.tile([P, CW], bf16, tag="ot", name="ot")
                        # half-width PSUM tiles (2 banks x 4 bufs) so the MM
                        # stream rotates into freed banks at a finer grain;
                        # post-op halves run on ACT || DVE concurrently
                        for s in range(2):
                            sl = slice(s * hw_, (s + 1) * hw_)
                            ps = psc.tile([P, hw_], f32, tag="ops",
                                          name="ops", bufs=4)
                            for q in range(hw_ // MMW):
                                qs = ch * CW + s * hw_ + q * MMW
                                for e in range(2):
                                    nc.tensor.matmul(
                                        ps[:, q * MMW:(q + 1) * MMW],
                                        mt_sb[e][:, ct * P:(ct + 1) * P],
                                        xn[e][:, qs:qs + MMW],
                                        start=(e == 0), stop=(e == 1),
                                    )
                            if s % 2 == 0:
                                nc.scalar.activation(
                                    out=ot[:, sl], in_=ps, func=Ident,
                                    bias=r_sb[ct], scale=rs_sb[ct])
                            else:
                                nc.vector.tensor_scalar(
                                    out=ot[:, sl], in0=ps,
                                    scalar1=rs_sb[ct], scalar2=r_sb[ct],
                                    op0=mult, op1=add)
                            if ch == NRES - 1:
                                nc.sync.dma_start(
                                    out=out_ap[ct * P:(ct + 1) * P,
                                               ch * CW + s * hw_:
                                               ch * CW + (s + 1) * hw_],
                                    in_=ot[:, sl])
                        if ch < NRES - 1:
                            nc.sync.dma_start(
                                out=out_ap[ct * P:(ct + 1) * P,
                                           ch * CW:(ch + 1) * CW],
                                in_=ot)

    nc.compile()
    return nc


def _get_nc():
    key = "v2"
    if key not in _BUILD_CACHE:
        _BUILD_CACHE[key] = _build()
    return _BUILD_CACHE[key]


def kernel(x, wq, bq, wk, bk, wv, bv):
    global LAST_RESULT
    from concourse.bass_utils import run_bass_kernel_spmd

    nc = _get_nc()

    x = np.ascontiguousarray(np.asarray(x, dtype=np.float32))
    xf = x.reshape(B, C, N)
    wq = np.ascontiguousarray(np.asarray(wq, dtype=np.float32))
    wk = np.ascontiguousarray(np.asarray(wk, dtype=np.float32))
    wv = np.ascontiguousarray(np.asarray(wv, dtype=np.float32))
    bq = np.ascontiguousarray(np.asarray(bq, dtype=np.float32))
    bk = np.ascontiguousarray(np.asarray(bk, dtype=np.float32))
    bv = np.ascontiguousarray(np.asarray(bv, dtype=np.float32))

    in_maps = _make_in_maps(xf, wq, bq, wk, bk, wv, bv)

    res = run_bass_kernel_spmd(nc, in_maps, core_ids=list(range(NCORES)))
    LAST_RESULT = res

    out = np.empty((B, C, N), dtype=np.float32)
    for i in range(NCORES):
        b, h = i // 2, i % 2
        out[b, :, h * HALF:(h + 1) * HALF] = np.asarray(
            res.results[i]["out"]).astype(np.float32)
    return out.reshape(B, C, W, H)


# ---------------------------------------------------------------------------
# Dev-loop helpers (not used by the grading path)
# ---------------------------------------------------------------------------

def timeline_ns():
    """Cost-model simulated duration of one core's program (ns)."""
    from concourse.timeline_sim import TimelineSim
    nc = _get_nc()
    ts = TimelineSim(nc)
    return ts.simulate()


def _make_in_maps(xf, wq, bq, wk, bk, wv, bv):
    ones_pad = np.zeros((N, 2), dtype=np.float32)
    ones_pad[:, 0] = 1.0
    wqwk = np.ascontiguousarray(np.concatenate([wq, wk], axis=0))
    vpk = np.concatenate(
        [wv, bv[:, None], np.zeros((C, 1), np.float32)],
        axis=1).astype(ml_dtypes.bfloat16)
    bpk = np.concatenate([bq, bk])[None, :].astype(np.float32)
    in_maps = []
    for i in range(NCORES):
        b, h = i // 2, i % 2
        xTb = np.concatenate([xf[b].T, ones_pad], axis=1)
        # own spatial half first: the kernel keeps the first NCHUNK/2 chunks
        # resident and derives its output columns from them
        xt = np.concatenate([xTb[h * HALF:(h + 1) * HALF],
                             xTb[(1 - h) * HALF:(2 - h) * HALF]],
                            axis=0).astype(ml_dtypes.bfloat16)
        xn = np.ascontiguousarray(
            xf[b][:, h * HALF + HALF // 2:(h + 1) * HALF]
        ).astype(ml_dtypes.bfloat16)
        in_maps.append({
            "xt": xt, "xn": xn,
            "wqwk": wqwk, "vpk": vpk, "bpk": bpk,
        })
    return in_maps


# revision 39
# speedup vs baseline: 1.2492x; 1.0103x over previous
"""Channel cross-attention kernel for Trainium2 (8 NeuronCores).

Math (exact restructuring of the reference):
    xf = x.reshape(B, C, N)
    q = wq xf + bq;  k = wk xf + bk;  v = wv xf + bv
    energy = q k^T = wq G wk^T + (wq sx) bk^T + bq (wk sx)^T + N bq bk^T
        where G = xf xf^T (C x C Gram), sx = xf @ 1 (row sums)
    att = softmax(energy / sqrt(N))
    out = att v + xf = (att wv) xf + (att bv) 1^T + xf = M xf + r 1^T + xf

Sharding: 8 cores, core i handles sample b=i//2, spatial half h=i%2.
Each core computes G over the FULL sample (redundantly within the pair, no
cross-core communication) and produces its own spatial half of the output.

Precision: x streams in bf16 (halves HBM read), out streams back bf16
(halves HBM write; host upcasts), everything between Gram and the output
matmul runs in f32(r). The Gram uses the symmetry G = G^T: the second
row-block only streams cols 128:258 and the missing 128x128 block is
reconstructed by one PE transpose.

Host prep per core: xt = xf[b].T (N, C+2: data, ones, pad) in bf16, rows
ordered own-spatial-half first. The Gram phase streams all rows; the own
half stays resident in SBUF and is transposed back on-chip for the output
phase.
"""

import os
import sys

for _p in ("/opt/trn_rl_repo", "/root/.axon_site/_ro/trn_rl_repo"):
    if os.path.isdir(_p) and _p not in sys.path:
        sys.path.append(_p)

import numpy as np
import ml_dtypes

# ---- problem constants (hardcoded; must match setup_inputs) ----
B, C, W, H = 4, 256, 128, 128
N = W * H            # 16384
HALF = N // 2        # 8192
P = 128              # partitions
NCORES = 8
SQRT_N = float(np.sqrt(N))   # 128.0
XT_COLS_D = C + 2            # xt DRAM row: 256 data cols, ones col, zero pad

GRAM_DT = "bf16"     # informational (printed by test harness)
MM_DT = "bf16"

_BUILD_CACHE = {}
LAST_RESULT = None   # BassKernelResults of the most recent run (for test harness)


def _build():
    import concourse.bacc as bacc
    import concourse.mybir as mybir
    import concourse.tile as tile
    from concourse.bass import MemorySpace
    from concourse.masks import make_identity

    f32 = mybir.dt.float32
    f32r = mybir.dt.float32r
    bf16 = mybir.dt.bfloat16

    nc = bacc.Bacc("TRN2", target_bir_lowering=False)

    xt_d = nc.dram_tensor("xt", (N, XT_COLS_D), bf16, kind="ExternalInput")
    # natural-layout own-half cols 4096:8192 (phase C chunks 2-3): loaded
    # directly instead of PE-transposing, filling the post-input DMA idle gap
    xn_d = nc.dram_tensor("xn", (C, HALF // 2), bf16, kind="ExternalInput")
    # host-packed weights: 3 single sync DMAs instead of 11 gpsimd ones
    # (SWDGE descriptor generation serializes ~1us each on Pool and lands
    # weights after the PE needs them)
    wqwk_d = nc.dram_tensor("wqwk", (4 * P, C), f32r, kind="ExternalInput")
    vpk_d = nc.dram_tensor("vpk", (2 * P, XT_COLS_D), bf16,
                           kind="ExternalInput")
    bpk_d = nc.dram_tensor("bpk", (1, 2 * C), f32r, kind="ExternalInput")
    out_d = nc.dram_tensor("out", (C, HALF), bf16, kind="ExternalOutput")

    xt_ap = xt_d.ap()
    out_ap = out_d.ap()

    NT = N // P          # 128 n-tiles for the Gram phase
    CH_T = 16            # n-tiles per SBUF tile
    NCHUNK = NT // CH_T  # 8 chunks
    Exp = mybir.ActivationFunctionType.Exp
    Copy = mybir.ActivationFunctionType.Copy
    Ident = mybir.ActivationFunctionType.Identity

    with tile.TileContext(nc) as tc:
        with (
            tc.tile_pool(name="singles", bufs=1) as singles,
            tc.tile_pool(name="work", bufs=2) as work,
        ):
            # ---------- constants ----------
            identity = singles.tile([P, P], f32, tag="ident", name="ident")
            make_identity(nc, identity)
            identity_r = singles.tile([P, P], bf16, tag="identr", name="identr")
            nc.vector.tensor_copy(out=identity_r, in_=identity)
            identity_fr = singles.tile([P, P], f32r, tag="identfr", name="identfr")
            nc.vector.tensor_copy(out=identity_fr, in_=identity)
            # warm the ACT Exp table early so phase B's exp doesn't pay the
            # ~1.3us table load on the critical path
            warm = singles.tile([1, 2], f32, tag="warm", name="warm")
            nc.vector.memset(warm, 0.0)
            nc.scalar.activation(out=warm, in_=warm, func=Exp,
                                 bias=0.0, scale=1.0)

            # weights natural layout, f32r so every phase-B matmul runs at
            # full PE rate; DMAs for these are issued inside the chunk loop
            # (after the first data chunk) to keep the data stream in front
            wqwk_sb = singles.tile([P, 4, C], f32r, tag="wqwk", name="wqwk")
            vpk_sb = singles.tile([P, 2, XT_COLS_D], bf16, tag="vpk",
                                  name="vpk")
            bpk_sb = singles.tile([1, 2 * C], f32r, tag="bpk", name="bpk")
            wq_sb = [wqwk_sb[:, j, :] for j in range(2)]
            wk_sb = [wqwk_sb[:, 2 + j, :] for j in range(2)]
            wv_sb = [vpk_sb[:, j, 0:C] for j in range(2)]
            bv_col = [vpk_sb[:, j, C:C + 1] for j in range(2)]
            bq_row = bpk_sb[:, 0:C]
            bk_row = bpk_sb[:, C:2 * C]
            bkN_row = singles.tile([1, C], f32r, tag="bknr", name="bknr")

            # transposed weights wqT[e][:, c] = wq[c, e], wkT likewise
            wqT_sb = [singles.tile([P, C], f32r, tag=f"wqT{j}", name=f"wqT{j}") for j in range(2)]
            wkT_sb = [singles.tile([P, C], f32r, tag=f"wkT{j}", name=f"wkT{j}") for j in range(2)]

            gsb = [singles.tile([P, C + 2], f32r, tag=f"gsb{m}", name=f"gsb{m}") for m in range(2)]
            t1sb = [singles.tile([P, C], f32r, tag=f"t1sb{m}", name=f"t1sb{m}") for m in range(2)]
            attT_sb = [singles.tile([P, C], bf16, tag=f"attT{m}", name=f"attT{m}") for m in range(2)]
            mt_sb = [singles.tile([P, C], bf16, tag=f"mt{m}", name=f"mt{m}") for m in range(2)]
            r_sb = [singles.tile([P, 1], f32, tag=f"r{m}", name=f"r{m}") for m in range(2)]
            sq_row = singles.tile([1, C], f32r, tag="sqr", name="sqr")
            sk_row = singles.tile([1, C], f32r, tag="skr", name="skr")
            ssum_sb = [singles.tile([P, 1], f32, tag=f"ssum{m}", name=f"ssum{m}") for m in range(2)]
            rs_sb = [singles.tile([P, 1], f32, tag=f"rs{m}", name=f"rs{m}") for m in range(2)]

            # ---------- phase A: Gram accumulation ----------
            # Symmetric Gram: row-block 0 streams all 258 cols; row-block 1
            # streams only cols 128:258 (G10 = G01^T is reconstructed by one
            # PE transpose afterwards). Own-half chunks are each PE-transposed
            # back to natural layout (xn) right after their DMA lands.
            NRES = NCHUNK // 2
            CW = CH_T * P     # chunk width in spatial cols (2048)
            TB = 4            # transposes batched per PSUM bank
            CH_T_S = 8        # n-tiles per streamed (non-resident) SBUF tile
            M1_LO, M1_W = P, XT_COLS_D - P   # cols 128:258 (130 wide)
            NTRES = NRES // 2  # chunks transposed on-chip (rest DMA'd via xn_d)
            xn = [singles.tile([P, HALF], bf16, tag=f"xn{m}",
                               name=f"xn{m}") for m in range(2)]
            with (
                tc.tile_pool(name="psg", bufs=1, space=MemorySpace.PSUM) as psg,
                tc.tile_pool(name="psct", bufs=5, space=MemorySpace.PSUM) as psct,
                tc.tile_pool(name="xtrp", bufs=3) as xtrp,
                tc.tile_pool(name="xtp", bufs=5) as xtp,
            ):
                g_ps0 = psg.tile([P, C + 2], f32, tag="g0", name="g0")
                g_ps1 = psg.tile([P, M1_W], f32, tag="g1", name="g1")
                # interleave own/streamed chunks so PE is never starved and
                # the DMA queue never runs dry
                jobs = [("own", 0), ("str", 0), ("own", 1), ("str", 1),
                        ("str", 2), ("own", 2), ("str", 3), ("str", 4),
                        ("own", 3), ("str", 5), ("str", 6), ("str", 7)]
                wjobs = [(wsrc, wdst, jj, ii)
                         for (wsrc, wdst) in ((wq_sb, wqT_sb), (wk_sb, wkT_sb))
                         for jj in range(2) for ii in range(2)]
                nt = 0
                ncopy = 0
                tqueue = []

                def emit_transposes(n):
                    nonlocal ncopy
                    for _ in range(n):
                        if not tqueue:
                            return
                        xtt, tch, m, tb = tqueue.pop(0)
                        tps = psct.tile([P, TB * P], bf16, tag="tps",
                                        name="tps")
                        for k in range(TB):
                            t = tb * TB + k
                            nc.tensor.transpose(
                                tps[:, k * P:(k + 1) * P],
                                xtt[:, t, m * P:(m + 1) * P],
                                identity_r)
                        dst = xn[m][:, tch * CW + tb * TB * P:
                                    tch * CW + (tb + 1) * TB * P]
                        if ncopy % 2 == 0:
                            nc.vector.tensor_copy(out=dst, in_=tps)
                        else:
                            nc.scalar.activation(out=dst, in_=tps,
                                                 func=Copy,
                                                 bias=0.0, scale=1.0)
                        ncopy += 1

                for ji, (kind, ch) in enumerate(jobs):
                    if kind == "own":
                        ctiles = CH_T
                        xt = xtrp.tile([P, CH_T, XT_COLS_D], bf16, tag="xtr",
                                       name="xtr")
                        if ch == 0:
                            # split the first chunk so PE starts ~4x sooner
                            QT = CH_T // 4
                            for q in range(4):
                                nc.sync.dma_start(
                                    out=xt[:, q * QT:(q + 1) * QT, :],
                                    in_=xt_ap[q * QT * P:(q + 1) * QT * P, :]
                                        .rearrange("(t p) c -> p t c", p=P),
                                )
                        else:
                            nc.sync.dma_start(
                                out=xt,
                                in_=xt_ap[ch * CH_T * P:(ch + 1) * CH_T * P, :]
                                    .rearrange("(t p) c -> p t c", p=P),
                            )
                    else:
                        ctiles = CH_T_S
                        xt = xtp.tile([P, CH_T_S, XT_COLS_D], bf16, tag="xt",
                                      name="xt")
                        row0 = NRES * CH_T * P + ch * CH_T_S * P
                        nc.sync.dma_start(
                            out=xt,
                            in_=xt_ap[row0:row0 + CH_T_S * P, :]
                                .rearrange("(t p) c -> p t c", p=P),
                        )
                    for t in range(ctiles):
                        nc.tensor.matmul(
                            g_ps0[:, 0:C + 2],
                            xt[:, t, 0:P],
                            xt[:, t, 0:C + 2],
                            start=(nt == 0), stop=(nt == NT - 1),
                        )
                        nc.tensor.matmul(
                            g_ps1[:, 0:M1_W],
                            xt[:, t, P:C],
                            xt[:, t, M1_LO:XT_COLS_D],
                            start=(nt == 0), stop=(nt == NT - 1),
                        )
                        nt += 1
                    if kind == "own" and ch < NTRES:
                        # queue this chunk's transposes; they are emitted a
                        # few groups per job as PE filler so DMA-paced gaps
                        # never idle the PE (idle resets the p-state ramp)
                        for m in range(2):
                            for tb in range(CH_T // TB):
                                tqueue.append((xt, ch, m, tb))
                    elif kind == "str" and ch < len(wjobs):
                        wsrc, wdst, jj, ii = wjobs[ch]
                        ps = psct.tile([P, P], f32r, tag="wt", name="wt",
                                       bufs=1)
                        nc.tensor.transpose(
                            ps, wsrc[ii][:, jj * P:(jj + 1) * P], identity_fr)
                        nc.vector.tensor_copy(
                            out=wdst[jj][:, ii * P:(ii + 1) * P], in_=ps)
                    if ji == 0:
                        # packed weights ride the same HWDGE queue right
                        # behind the first data chunk
                        nc.sync.dma_start(
                            out=wqwk_sb,
                            in_=wqwk_d.ap().rearrange("(j p) c -> p j c", p=P))
                        nc.sync.dma_start(
                            out=vpk_sb,
                            in_=vpk_d.ap().rearrange("(j p) c -> p j c", p=P))
                        nc.sync.dma_start(out=bpk_sb, in_=bpk_d.ap())
                        nc.vector.tensor_scalar_mul(bkN_row, bk_row, float(N))
                    else:
                        emit_transposes(2)
                emit_transposes(len(tqueue))

                # natural-layout chunks 2-3 arrive by DMA in the gap after
                # the Gram input finishes (queued behind all xt loads)
                for m in range(2):
                    nc.sync.dma_start(
                        out=xn[m][:, NTRES * CW:HALF],
                        in_=xn_d.ap()[m * P:(m + 1) * P, :])

                # G to SBUF: block row 0 fully (split across DVE/ACT; the
                # 128:258 slice lands first so the fixup transpose can start),
                # block row 1 cols 128:258 from PSUM via ACT
                nc.vector.tensor_copy(out=gsb[0][:, P:XT_COLS_D],
                                      in_=g_ps0[:, P:XT_COLS_D])
                nc.scalar.activation(out=gsb[0][:, 0:P], in_=g_ps0[:, 0:P],
                                     func=Copy, bias=0.0, scale=1.0)
                nc.scalar.activation(out=gsb[1][:, M1_LO:XT_COLS_D],
                                     in_=g_ps1, func=Copy, bias=0.0, scale=1.0)

            # ---------- phase B: energy^T, exp, M, r ----------
            # Everything is computed directly in the TRANSPOSED (d, c)
            # orientation (G is symmetric), so no PE<->DVE transpose
            # ping-pong. exp is taken without max-subtraction (energies
            # here are |e|/sqrt(N) < ~50, exp < 1e20, far from fp32
            # overflow); the 1/rowsum normalization is folded into the
            # phase-C output scale.
            with tc.tile_pool(name="psb", bufs=1, space=MemorySpace.PSUM) as psb:
                # G fixup: G10 = (G01)^T via one PE transpose
                tfix = psb.tile([P, P], f32r, tag="wt", name="tfix", bufs=1)
                nc.tensor.transpose(tfix, gsb[0][:, P:C], identity_fr)
                nc.vector.tensor_copy(out=gsb[1][:, 0:P], in_=tfix)

                # sq' = wq sx (row), sk' = wk sx (row) — early, they feed the
                # rank-1 energy terms; copies split ACT/DVE
                sq_ps = psb.tile([1, C], f32, tag="rps", name="sqp", bufs=2)
                for e in range(2):
                    nc.tensor.matmul(
                        sq_ps, gsb[e][:, C:C + 1], wqT_sb[e][:, 0:C],
                        start=(e == 0), stop=(e == 1))
                nc.scalar.activation(out=sq_row, in_=sq_ps, func=Copy,
                                     bias=0.0, scale=1.0)
                sk_ps = psb.tile([1, C], f32, tag="rps", name="skp", bufs=2)
                for e in range(2):
                    nc.tensor.matmul(
                        sk_ps, gsb[e][:, C:C + 1], wkT_sb[e][:, 0:C],
                        start=(e == 0), stop=(e == 1))
                nc.vector.tensor_copy(out=sk_row, in_=sk_ps)

                # T1q[a, c] = (G wq^T)[a, c]; e=0 contributions first so they
                # run while the G fixup completes
                t1_ps = [psb.tile([P, C], f32, tag="tmp", name=f"t1p{a}",
                                  bufs=3) for a in range(2)]
                for a in range(2):
                    nc.tensor.matmul(
                        t1_ps[a], gsb[0][:, a * P:(a + 1) * P],
                        wqT_sb[0][:, 0:C], start=True, stop=False)
                for a in range(2):
                    nc.tensor.matmul(
                        t1_ps[a], gsb[1][:, a * P:(a + 1) * P],
                        wqT_sb[1][:, 0:C], start=False, stop=True)
                nc.vector.tensor_copy(out=t1sb[0], in_=t1_ps[0])
                nc.scalar.activation(out=t1sb[1], in_=t1_ps[1], func=Copy,
                                     bias=0.0, scale=1.0)

                # energyT (raw, unscaled) per d-tile:
                # energyT[d, c] = (wk G wq^T)[d, c] + bk[d] sq'[c]
                #                 + sk'[d] bq[c] + N bk[d] bq[c]
                eT_ps = [psb.tile([P, C], f32, tag=f"eps{dt}", name=f"eps{dt}")
                         for dt in range(2)]
                for dt in range(2):
                    ds_ = (dt * P, (dt + 1) * P)
                    nc.tensor.matmul(
                        eT_ps[dt],
                        wkT_sb[0][:, dt * P:(dt + 1) * P],
                        t1sb[0][:, 0:C],
                        start=True, stop=False,
                    )
                    nc.tensor.matmul(eT_ps[dt], bkN_row[:, ds_[0]:ds_[1]],
                                     bq_row[:, 0:C], start=False, stop=False)
                    nc.tensor.matmul(eT_ps[dt], bk_row[:, ds_[0]:ds_[1]],
                                     sq_row[:, 0:C], start=False, stop=False)
                    nc.tensor.matmul(
                        eT_ps[dt],
                        wkT_sb[1][:, dt * P:(dt + 1) * P],
                        t1sb[1][:, 0:C],
                        start=False, stop=False,
                    )
                    nc.tensor.matmul(eT_ps[dt], sk_row[:, ds_[0]:ds_[1]],
                                     bq_row[:, 0:C], start=False, stop=True)
                    # attT (unnormalized): exp(energyT / sqrt(N))
                    nc.scalar.activation(
                        out=attT_sb[dt], in_=eT_ps[dt], func=Exp,
                        bias=0.0, scale=1.0 / SQRT_N)

                ones_col = singles.tile([P, 1], bf16, tag="ones", name="ones")
                nc.vector.memset(ones_col, 1.0)

                # row sums: ssum[c] = sum_d attT[d, c] (column via matmul)
                dg_bf = [work.tile([P, P], bf16, tag=f"dg{e}", name=f"dg{e}")
                         for e in range(2)]
                for ct in range(2):
                    ps = psb.tile([P, 1], f32, tag="rps", name="rps", bufs=2)
                    for d in range(2):
                        nc.tensor.matmul(
                            ps, attT_sb[d][:, ct * P:(ct + 1) * P], ones_col,
                            start=(d == 0), stop=(d == 1))
                    nc.vector.tensor_copy(out=ssum_sb[ct], in_=ps)
                    nc.vector.reciprocal(out=rs_sb[ct], in_=ssum_sb[ct])
                    # diag(ssum) as bf16 for the in-PSUM diagonal matmul
                    nc.vector.tensor_scalar_mul(dg_bf[ct], identity_r,
                                                ssum_sb[ct])

                # MT[e][:, c] = M~[c, e] = sum_d att~[c, d] wv[d, e]
                # (+ diag(ssum) added by a PE matmul so phase C's rs scale
                # yields M x + x); copies split DVE/ACT
                for e in range(2):
                    ps = psb.tile([P, C], f32, tag="tmp", name=f"mtp{e}",
                                  bufs=3)
                    for d in range(2):
                        nc.tensor.matmul(
                            ps,
                            wv_sb[d][:, e * P:(e + 1) * P],
                            attT_sb[d][:, 0:C],
                            start=(d == 0), stop=False,
                        )
                    nc.tensor.matmul(
                        ps[:, e * P:(e + 1) * P], dg_bf[e], identity_r,
                        start=False, stop=True)
                    if e == 0:
                        nc.vector.tensor_copy(out=mt_sb[e], in_=ps)
                    else:
                        nc.scalar.activation(out=mt_sb[e], in_=ps, func=Copy,
                                             bias=0.0, scale=1.0)

                # r[c] = rs[c] * sum_d att~[c, d] bv[d]
                for ct in range(2):
                    ps = psb.tile([P, 1], f32, tag="rps", name="rps", bufs=2)
                    for d in range(2):
                        nc.tensor.matmul(
                            ps, attT_sb[d][:, ct * P:(ct + 1) * P], bv_col[d],
                            start=(d == 0), stop=(d == 1))
                    nc.vector.tensor_copy(out=r_sb[ct], in_=ps)
                    nc.vector.tensor_mul(r_sb[ct], r_sb[ct], rs_sb[ct])

            # ---------- phase C: out = (M + I) x + r ----------
            MMW = 512         # matmul free width (one PSUM bank of f32)
            mult = mybir.AluOpType.mult
            add = mybir.AluOpType.add
            with (
                tc.tile_pool(name="psc", bufs=2, space=MemorySpace.PSUM) as psc,
                tc.tile_pool(name="outp", bufs=3) as outp,
            ):
                hw_ = CW // 2
                for ch in range(NRES):
                    for ct in range(2):
                        ot = outp.tile([P, CW], bf16, tag="ot", name="ot")
                        # half-width PSUM tiles (2 banks x 4 bufs) so the MM
                        # stream rotates into freed banks at a finer grain;
                        # post-op halves run on ACT || DVE concurrently
                        for s in range(2):
                            sl = slice(s * hw_, (s + 1) * hw_)
                            ps = psc.tile([P, hw_], f32, tag="ops",
                                          name="ops", bufs=4)
                            for q in range(hw_ // MMW):
                                qs = ch * CW + s * hw_ + q * MMW
                                for e in range(2):
                                    nc.tensor.matmul(
                                        ps[:, q * MMW:(q + 1) * MMW],
                                        mt_sb[e][:, ct * P:(ct + 1) * P],
                                        xn[e][:, qs:qs + MMW],
                                        start=(e == 0), stop=(e == 1),
                                    )
                            if s % 2 == 0:
                                nc.scalar.activation(
                                    out=ot[:, sl], in_=ps, func=Ident,
                                    bias=r_sb[ct], scale=rs_sb[ct])
                            else:
                                nc.vector.tensor_scalar(
                                    out=ot[:, sl], in0=ps,
                                    scalar1=rs_sb[ct], scalar2=r_sb[ct],
                                    op0=mult, op1=add)
                            if ch == NRES - 1:
                                nc.sync.dma_start(
                                    out=out_ap[ct * P:(ct + 1) * P,
                                               ch * CW + s * hw_:
                                               ch * CW + (s + 1) * hw_],
                                    in_=ot[:, sl])
                        if ch < NRES - 1:
                            nc.sync.dma_start(
                                out=out_ap[ct * P:(ct + 1) * P,
                                           ch * CW:(ch + 1) * CW],
                                in_=ot)

    nc.compile()
    return nc


def _get_nc():
    key = "v2"
    if key not in _BUILD_CACHE:
        _BUILD_CACHE[key] = _build()
    return _BUILD_CACHE[key]


def kernel(x, wq, bq, wk, bk, wv, bv):
    global LAST_RESULT
    from concourse.bass_utils import run_bass_kernel_spmd

    nc = _get_nc()

    x = np.ascontiguousarray(np.asarray(x, dtype=np.float32))
    xf = x.reshape(B, C, N)
    wq = np.ascontiguousarray(np.asarray(wq, dtype=np.float32))
    wk = np.ascontiguousarray(np.asarray(wk, dtype=np.float32))
    wv = np.ascontiguousarray(np.asarray(wv, dtype=np.float32))
    bq = np.ascontiguousarray(np.asarray(bq, dtype=np.float32))
    bk = np.ascontiguousarray(np.asarray(bk, dtype=np.float32))
    bv = np.ascontiguousarray(np.asarray(bv, dtype=np.float32))

    in_maps = _make_in_maps(xf, wq, bq, wk, bk, wv, bv)

    res = run_bass_kernel_spmd(nc, in_maps, core_ids=list(range(NCORES)))
    LAST_RESULT = res

    out = np.empty((B, C, N), dtype=np.float32)
    for i in range(NCORES):
        b, h = i // 2, i % 2
        out[b, :, h * HALF:(h + 1) * HALF] = np.asarray(
            res.results[i]["out"]).astype(np.float32)
    return out.reshape(B, C, W, H)


# ---------------------------------------------------------------------------
# Dev-loop helpers (not used by the grading path)
# ---------------------------------------------------------------------------

def timeline_ns():
    """Cost-model simulated duration of one core's program (ns)."""
    from concourse.timeline_sim import TimelineSim
    nc = _get_nc()
    ts = TimelineSim(nc)
    return ts.simulate()


def _make_in_maps(xf, wq, bq, wk, bk, wv, bv):
    ones_pad = np.zeros((N, 2), dtype=np.float32)
    ones_pad[:, 0] = 1.0
    wqwk = np.ascontiguousarray(np.concatenate([wq, wk], axis=0))
    vpk = np.concatenate(
        [wv, bv[:, None], np.zeros((C, 1), np.float32)],
        axis=1).astype(ml_dtypes.bfloat16)
    bpk = np.concatenate([bq, bk])[None, :].astype(np.float32)
    in_maps = []
    for i in range(NCORES):
        b, h = i // 2, i % 2
        xTb = np.concatenate([xf[b].T, ones_pad], axis=1)
        # own spatial half first: the kernel keeps the first NCHUNK/2 chunks
        # resident and derives its output columns from them
        xt = np.concatenate([xTb[h * HALF:(h + 1) * HALF],
                             xTb[(1 - h) * HALF:(2 - h) * HALF]],
                            axis=0).astype(ml_dtypes.bfloat16)
        xn = np.ascontiguousarray(
            xf[b][:, h * HALF + HALF // 2:(h + 1) * HALF]
        ).astype(ml_dtypes.bfloat16)
        in_maps.append({
            "xt": xt, "xn": xn,
            "wqwk": wqwk, "vpk": vpk, "bpk": bpk,
        })
    return in_maps


# revision 44
# speedup vs baseline: 1.3198x; 1.0565x over previous
"""Channel cross-attention kernel for Trainium2 (8 NeuronCores).

Math (exact restructuring of the reference):
    xf = x.reshape(B, C, N)
    q = wq xf + bq;  k = wk xf + bk;  v = wv xf + bv
    energy = q k^T = wq G wk^T + (wq sx) bk^T + bq (wk sx)^T + N bq bk^T
        where G = xf xf^T (C x C Gram), sx = xf @ 1 (row sums)
    att = softmax(energy / sqrt(N))
    out = att v + xf = (att wv) xf + (att bv) 1^T + xf = M xf + r 1^T + xf

Sharding: 8 cores, core i handles sample b=i//2, spatial half h=i%2.
Each core computes G over the FULL sample (redundantly within the pair, no
cross-core communication) and produces its own spatial half of the output.

Precision: x streams in bf16 (halves HBM read), out streams back bf16
(halves HBM write; host upcasts), everything between Gram and the output
matmul runs in f32(r). The Gram uses the symmetry G = G^T: the second
row-block only streams cols 128:258 and the missing 128x128 block is
reconstructed by one PE transpose.

Host prep per core: xt = xf[b].T (N, C+2: data, ones, pad) in bf16, rows
ordered own-spatial-half first. The Gram phase streams all rows; the own
half stays resident in SBUF and is transposed back on-chip for the output
phase.
"""

import os
import sys

for _p in ("/opt/trn_rl_repo", "/root/.axon_site/_ro/trn_rl_repo"):
    if os.path.isdir(_p) and _p not in sys.path:
        sys.path.append(_p)

import numpy as np
import ml_dtypes

# ---- problem constants (hardcoded; must match setup_inputs) ----
B, C, W, H = 4, 256, 128, 128
N = W * H            # 16384
HALF = N // 2        # 8192
P = 128              # partitions
NCORES = 8
SQRT_N = float(np.sqrt(N))   # 128.0
XT_COLS_D = C + 2            # xt DRAM row: 256 data cols, ones col, zero pad

GRAM_DT = "bf16"     # informational (printed by test harness)
MM_DT = "bf16"

_BUILD_CACHE = {}
LAST_RESULT = None   # BassKernelResults of the most recent run (for test harness)


def _build():
    import concourse.bacc as bacc
    import concourse.mybir as mybir
    import concourse.tile as tile
    from concourse.bass import MemorySpace
    from concourse.masks import make_identity

    f32 = mybir.dt.float32
    f32r = mybir.dt.float32r
    bf16 = mybir.dt.bfloat16

    nc = bacc.Bacc("TRN2", target_bir_lowering=False)

    xt_d = nc.dram_tensor("xt", (N, XT_COLS_D), bf16, kind="ExternalInput")
    # natural-layout own-half cols 4096:8192 (phase C chunks 2-3): loaded
    # directly instead of PE-transposing, filling the post-input DMA idle gap
    xn_d = nc.dram_tensor("xn", (C, HALF // 2), bf16, kind="ExternalInput")
    # host-packed weights: 3 single sync DMAs instead of 11 gpsimd ones
    # (SWDGE descriptor generation serializes ~1us each on Pool and lands
    # weights after the PE needs them)
    wqwk_d = nc.dram_tensor("wqwk", (4 * P, C), f32r, kind="ExternalInput")
    vpk_d = nc.dram_tensor("vpk", (2 * P, XT_COLS_D), bf16,
                           kind="ExternalInput")
    bpk_d = nc.dram_tensor("bpk", (1, 2 * C), f32r, kind="ExternalInput")
    out_d = nc.dram_tensor("out", (C, HALF), bf16, kind="ExternalOutput")

    xt_ap = xt_d.ap()
    out_ap = out_d.ap()

    NT = N // P          # 128 n-tiles for the Gram phase
    CH_T = 16            # n-tiles per SBUF tile
    NCHUNK = NT // CH_T  # 8 chunks
    Exp = mybir.ActivationFunctionType.Exp
    Copy = mybir.ActivationFunctionType.Copy
    Ident = mybir.ActivationFunctionType.Identity

    with tile.TileContext(nc) as tc:
        with (
            tc.tile_pool(name="singles", bufs=1) as singles,
            tc.tile_pool(name="work", bufs=2) as work,
        ):
            # ---------- constants ----------
            identity = singles.tile([P, P], f32, tag="ident", name="ident")
            make_identity(nc, identity)
            identity_r = singles.tile([P, P], bf16, tag="identr", name="identr")
            nc.vector.tensor_copy(out=identity_r, in_=identity)
            identity_fr = singles.tile([P, P], f32r, tag="identfr", name="identfr")
            nc.vector.tensor_copy(out=identity_fr, in_=identity)
            # warm the ACT Exp table early so phase B's exp doesn't pay the
            # ~1.3us table load on the critical path
            warm = singles.tile([1, 2], f32, tag="warm", name="warm")
            nc.vector.memset(warm, 0.0)
            nc.scalar.activation(out=warm, in_=warm, func=Exp,
                                 bias=0.0, scale=1.0)


            # weights natural layout, f32r so every phase-B matmul runs at
            # full PE rate; DMAs for these are issued inside the chunk loop
            # (after the first data chunk) to keep the data stream in front
            wqwk_sb = singles.tile([P, 4, C], f32r, tag="wqwk", name="wqwk")
            vpk_sb = singles.tile([P, 2, XT_COLS_D], bf16, tag="vpk",
                                  name="vpk")
            bpk_sb = singles.tile([1, 2 * C], f32r, tag="bpk", name="bpk")
            wq_sb = [wqwk_sb[:, j, :] for j in range(2)]
            wk_sb = [wqwk_sb[:, 2 + j, :] for j in range(2)]
            wv_sb = [vpk_sb[:, j, 0:C] for j in range(2)]
            bv_col = [vpk_sb[:, j, C:C + 1] for j in range(2)]
            bq_row = bpk_sb[:, 0:C]
            bk_row = bpk_sb[:, C:2 * C]
            bkN_row = singles.tile([1, C], f32r, tag="bknr", name="bknr")

            # transposed weights wqT[e][:, c] = wq[c, e], wkT likewise
            wqT_sb = [singles.tile([P, C], f32r, tag=f"wqT{j}", name=f"wqT{j}") for j in range(2)]
            wkT_sb = [singles.tile([P, C], f32r, tag=f"wkT{j}", name=f"wkT{j}") for j in range(2)]

            gsb = [singles.tile([P, C + 2], f32r, tag=f"gsb{m}", name=f"gsb{m}") for m in range(2)]
            t1sb = [singles.tile([P, C], f32r, tag=f"t1sb{m}", name=f"t1sb{m}") for m in range(2)]
            attT_sb = [singles.tile([P, C], bf16, tag=f"attT{m}", name=f"attT{m}") for m in range(2)]
            mt_sb = [singles.tile([P, C], bf16, tag=f"mt{m}", name=f"mt{m}") for m in range(2)]
            r_sb = [singles.tile([P, 1], f32, tag=f"r{m}", name=f"r{m}") for m in range(2)]
            sq_row = singles.tile([1, C], f32r, tag="sqr", name="sqr")
            sk_row = singles.tile([1, C], f32r, tag="skr", name="skr")
            ssum_sb = [singles.tile([P, 1], f32, tag=f"ssum{m}", name=f"ssum{m}") for m in range(2)]
            rs_sb = [singles.tile([P, 1], f32, tag=f"rs{m}", name=f"rs{m}") for m in range(2)]

            # ---------- phase A: Gram accumulation ----------
            # Symmetric Gram: row-block 0 streams all 258 cols; row-block 1
            # streams only cols 128:258 (G10 = G01^T is reconstructed by one
            # PE transpose afterwards). Own-half chunks are each PE-transposed
            # back to natural layout (xn) right after their DMA lands.
            NRES = NCHUNK // 2
            CW = CH_T * P     # chunk width in spatial cols (2048)
            TB = 4            # transposes batched per PSUM bank
            CH_T_S = 8        # n-tiles per streamed (non-resident) SBUF tile
            M1_LO, M1_W = P, XT_COLS_D - P   # cols 128:258 (130 wide)
            NTRES = NRES // 2  # chunks transposed on-chip (rest DMA'd via xn_d)
            xn = [singles.tile([P, HALF], bf16, tag=f"xn{m}",
                               name=f"xn{m}") for m in range(2)]
            with (
                tc.tile_pool(name="psg", bufs=1, space=MemorySpace.PSUM) as psg,
                tc.tile_pool(name="psct", bufs=5, space=MemorySpace.PSUM) as psct,
                tc.tile_pool(name="xtrp", bufs=3) as xtrp,
                tc.tile_pool(name="xtp", bufs=5) as xtp,
            ):
                g_ps0 = psg.tile([P, C + 2], f32, tag="g0", name="g0")
                g_ps1 = psg.tile([P, M1_W], f32, tag="g1", name="g1")
                # PE prewarm: dummy transposes of the identity keep the PE
                # busy while the first data chunk streams in, so the p-state
                # ramp completes before the Gram matmuls start (PE idle
                # resets the ramp to half clock)
                wps = psct.tile([P, P], bf16, tag="tps", name="warmt")
                for _ in range(14):
                    nc.tensor.transpose(wps, identity_r, identity_r)
                # interleave own/streamed chunks so PE is never starved and
                # the DMA queue never runs dry
                jobs = [("own", 0), ("str", 0), ("own", 1), ("str", 1),
                        ("str", 2), ("own", 2), ("str", 3), ("str", 4),
                        ("own", 3), ("str", 5), ("str", 6), ("str", 7)]
                wjobs = [(wsrc, wdst, jj, ii)
                         for (wsrc, wdst) in ((wq_sb, wqT_sb), (wk_sb, wkT_sb))
                         for jj in range(2) for ii in range(2)]
                nt = 0
                ncopy = 0
                tqueue = []

                def emit_transposes(n):
                    nonlocal ncopy
                    for _ in range(n):
                        if not tqueue:
                            return
                        xtt, tch, m, tb = tqueue.pop(0)
                        tps = psct.tile([P, TB * P], bf16, tag="tps",
                                        name="tps")
                        for k in range(TB):
                            t = tb * TB + k
                            nc.tensor.transpose(
                                tps[:, k * P:(k + 1) * P],
                                xtt[:, t, m * P:(m + 1) * P],
                                identity_r)
                        dst = xn[m][:, tch * CW + tb * TB * P:
                                    tch * CW + (tb + 1) * TB * P]
                        if ncopy % 2 == 0:
                            nc.vector.tensor_copy(out=dst, in_=tps)
                        else:
                            nc.scalar.activation(out=dst, in_=tps,
                                                 func=Copy,
                                                 bias=0.0, scale=1.0)
                        ncopy += 1

                for ji, (kind, ch) in enumerate(jobs):
                    if kind == "own":
                        ctiles = CH_T
                        xt = xtrp.tile([P, CH_T, XT_COLS_D], bf16, tag="xtr",
                                       name="xtr")
                        if ch == 0:
                            # split the first chunk so PE starts ~4x sooner
                            QT = CH_T // 4
                            for q in range(4):
                                nc.sync.dma_start(
                                    out=xt[:, q * QT:(q + 1) * QT, :],
                                    in_=xt_ap[q * QT * P:(q + 1) * QT * P, :]
                                        .rearrange("(t p) c -> p t c", p=P),
                                )
                        else:
                            nc.sync.dma_start(
                                out=xt,
                                in_=xt_ap[ch * CH_T * P:(ch + 1) * CH_T * P, :]
                                    .rearrange("(t p) c -> p t c", p=P),
                            )
                    else:
                        ctiles = CH_T_S
                        xt = xtp.tile([P, CH_T_S, XT_COLS_D], bf16, tag="xt",
                                      name="xt")
                        row0 = NRES * CH_T * P + ch * CH_T_S * P
                        nc.sync.dma_start(
                            out=xt,
                            in_=xt_ap[row0:row0 + CH_T_S * P, :]
                                .rearrange("(t p) c -> p t c", p=P),
                        )
                    for t in range(ctiles):
                        nc.tensor.matmul(
                            g_ps0[:, 0:C + 2],
                            xt[:, t, 0:P],
                            xt[:, t, 0:C + 2],
                            start=(nt == 0), stop=(nt == NT - 1),
                        )
                        nc.tensor.matmul(
                            g_ps1[:, 0:M1_W],
                            xt[:, t, P:C],
                            xt[:, t, M1_LO:XT_COLS_D],
                            start=(nt == 0), stop=(nt == NT - 1),
                        )
                        nt += 1
                    if kind == "own" and ch < NTRES:
                        # queue this chunk's transposes; they are emitted a
                        # few groups per job as PE filler so DMA-paced gaps
                        # never idle the PE (idle resets the p-state ramp)
                        for m in range(2):
                            for tb in range(CH_T // TB):
                                tqueue.append((xt, ch, m, tb))
                    elif kind == "str" and ch < len(wjobs):
                        wsrc, wdst, jj, ii = wjobs[ch]
                        ps = psct.tile([P, P], f32r, tag="wt", name="wt",
                                       bufs=1)
                        nc.tensor.transpose(
                            ps, wsrc[ii][:, jj * P:(jj + 1) * P], identity_fr)
                        nc.vector.tensor_copy(
                            out=wdst[jj][:, ii * P:(ii + 1) * P], in_=ps)
                    if ji == 0:
                        # packed weights ride the same HWDGE queue right
                        # behind the first data chunk
                        nc.sync.dma_start(
                            out=wqwk_sb,
                            in_=wqwk_d.ap().rearrange("(j p) c -> p j c", p=P))
                        nc.sync.dma_start(
                            out=vpk_sb,
                            in_=vpk_d.ap().rearrange("(j p) c -> p j c", p=P))
                        nc.sync.dma_start(out=bpk_sb, in_=bpk_d.ap())
                        nc.vector.tensor_scalar_mul(bkN_row, bk_row, float(N))
                    elif ji < 3:
                        emit_transposes(1)
                    else:
                        emit_transposes(3)
                emit_transposes(len(tqueue))

                # natural-layout chunks 2-3 arrive by DMA in the gap after
                # the Gram input finishes (queued behind all xt loads)
                for m in range(2):
                    nc.sync.dma_start(
                        out=xn[m][:, NTRES * CW:HALF],
                        in_=xn_d.ap()[m * P:(m + 1) * P, :])

                # G to SBUF: block row 0 fully (split across DVE/ACT; the
                # 128:258 slice lands first so the fixup transpose can start),
                # block row 1 cols 128:258 from PSUM via ACT
                nc.vector.tensor_copy(out=gsb[0][:, P:XT_COLS_D],
                                      in_=g_ps0[:, P:XT_COLS_D])
                nc.scalar.activation(out=gsb[0][:, 0:P], in_=g_ps0[:, 0:P],
                                     func=Copy, bias=0.0, scale=1.0)
                nc.scalar.activation(out=gsb[1][:, M1_LO:XT_COLS_D],
                                     in_=g_ps1, func=Copy, bias=0.0, scale=1.0)

            # ---------- phase B: energy^T, exp, M, r ----------
            # Everything is computed directly in the TRANSPOSED (d, c)
            # orientation (G is symmetric), so no PE<->DVE transpose
            # ping-pong. exp is taken without max-subtraction (energies
            # here are |e|/sqrt(N) < ~50, exp < 1e20, far from fp32
            # overflow); the 1/rowsum normalization is folded into the
            # phase-C output scale.
            with tc.tile_pool(name="psb", bufs=1, space=MemorySpace.PSUM) as psb:
                # G fixup: G10 = (G01)^T via one PE transpose
                tfix = psb.tile([P, P], f32r, tag="wt", name="tfix", bufs=1)
                nc.tensor.transpose(tfix, gsb[0][:, P:C], identity_fr)
                nc.vector.tensor_copy(out=gsb[1][:, 0:P], in_=tfix)

                # sq' = wq sx (row), sk' = wk sx (row) — early, they feed the
                # rank-1 energy terms; copies split ACT/DVE
                sq_ps = psb.tile([1, C], f32, tag="rps", name="sqp", bufs=2)
                for e in range(2):
                    nc.tensor.matmul(
                        sq_ps, gsb[e][:, C:C + 1], wqT_sb[e][:, 0:C],
                        start=(e == 0), stop=(e == 1))
                nc.scalar.activation(out=sq_row, in_=sq_ps, func=Copy,
                                     bias=0.0, scale=1.0)
                sk_ps = psb.tile([1, C], f32, tag="rps", name="skp", bufs=2)
                for e in range(2):
                    nc.tensor.matmul(
                        sk_ps, gsb[e][:, C:C + 1], wkT_sb[e][:, 0:C],
                        start=(e == 0), stop=(e == 1))
                nc.vector.tensor_copy(out=sk_row, in_=sk_ps)

                # T1q[a, c] = (G wq^T)[a, c]; e=0 contributions first so they
                # run while the G fixup completes
                t1_ps = [psb.tile([P, C], f32, tag="tmp", name=f"t1p{a}",
                                  bufs=3) for a in range(2)]
                for a in (1, 0):   # a=1 only needs the DVE half of gsb[0]
                    nc.tensor.matmul(
                        t1_ps[a], gsb[0][:, a * P:(a + 1) * P],
                        wqT_sb[0][:, 0:C], start=True, stop=False)
                for a in (1, 0):
                    nc.tensor.matmul(
                        t1_ps[a], gsb[1][:, a * P:(a + 1) * P],
                        wqT_sb[1][:, 0:C], start=False, stop=True)
                nc.vector.tensor_copy(out=t1sb[0], in_=t1_ps[0])
                nc.scalar.activation(out=t1sb[1], in_=t1_ps[1], func=Copy,
                                     bias=0.0, scale=1.0)

                # energyT (raw, unscaled) per d-tile:
                # energyT[d, c] = (wk G wq^T)[d, c] + bk[d] sq'[c]
                #                 + sk'[d] bq[c] + N bk[d] bq[c]
                eT_ps = [psb.tile([P, C], f32, tag=f"eps{dt}", name=f"eps{dt}")
                         for dt in range(2)]
                for dt in range(2):
                    ds_ = (dt * P, (dt + 1) * P)
                    nc.tensor.matmul(
                        eT_ps[dt],
                        wkT_sb[0][:, dt * P:(dt + 1) * P],
                        t1sb[0][:, 0:C],
                        start=True, stop=False,
                    )
                    nc.tensor.matmul(eT_ps[dt], bkN_row[:, ds_[0]:ds_[1]],
                                     bq_row[:, 0:C], start=False, stop=False)
                    nc.tensor.matmul(eT_ps[dt], bk_row[:, ds_[0]:ds_[1]],
                                     sq_row[:, 0:C], start=False, stop=False)
                    nc.tensor.matmul(
                        eT_ps[dt],
                        wkT_sb[1][:, dt * P:(dt + 1) * P],
                        t1sb[1][:, 0:C],
                        start=False, stop=False,
                    )
                    nc.tensor.matmul(eT_ps[dt], sk_row[:, ds_[0]:ds_[1]],
                                     bq_row[:, 0:C], start=False, stop=True)
                    # attT (unnormalized): exp(energyT / sqrt(N))
                    nc.scalar.activation(
                        out=attT_sb[dt], in_=eT_ps[dt], func=Exp,
                        bias=0.0, scale=1.0 / SQRT_N)

                ones_col = singles.tile([P, 1], bf16, tag="ones", name="ones")
                nc.vector.memset(ones_col, 1.0)

                # row sums: ssum[c] = sum_d attT[d, c] (column via matmul)
                dg_bf = [work.tile([P, P], bf16, tag=f"dg{e}", name=f"dg{e}")
                         for e in range(2)]
                for ct in range(2):
                    ps = psb.tile([P, 1], f32, tag="rps", name="rps", bufs=2)
                    for d in range(2):
                        nc.tensor.matmul(
                            ps, attT_sb[d][:, ct * P:(ct + 1) * P], ones_col,
                            start=(d == 0), stop=(d == 1))
                    nc.vector.tensor_copy(out=ssum_sb[ct], in_=ps)
                    nc.vector.reciprocal(out=rs_sb[ct], in_=ssum_sb[ct])
                    # diag(ssum) as bf16 for the in-PSUM diagonal matmul
                    nc.vector.tensor_scalar_mul(dg_bf[ct], identity_r,
                                                ssum_sb[ct])

                # MT[e][:, c] = M~[c, e] = sum_d att~[c, d] wv[d, e]
                # (+ diag(ssum) added by a PE matmul so phase C's rs scale
                # yields M x + x); copies split DVE/ACT
                for e in range(2):
                    ps = psb.tile([P, C], f32, tag="tmp", name=f"mtp{e}",
                                  bufs=3)
                    for d in range(2):
                        nc.tensor.matmul(
                            ps,
                            wv_sb[d][:, e * P:(e + 1) * P],
                            attT_sb[d][:, 0:C],
                            start=(d == 0), stop=False,
                        )
                    nc.tensor.matmul(
                        ps[:, e * P:(e + 1) * P], dg_bf[e], identity_r,
                        start=False, stop=True)
                    if e == 0:
                        nc.vector.tensor_copy(out=mt_sb[e], in_=ps)
                    else:
                        nc.scalar.activation(out=mt_sb[e], in_=ps, func=Copy,
                                             bias=0.0, scale=1.0)

                # r[c] = rs[c] * sum_d att~[c, d] bv[d]
                for ct in range(2):
                    ps = psb.tile([P, 1], f32, tag="rps", name="rps", bufs=2)
                    for d in range(2):
                        nc.tensor.matmul(
                            ps, attT_sb[d][:, ct * P:(ct + 1) * P], bv_col[d],
                            start=(d == 0), stop=(d == 1))
                    nc.vector.tensor_copy(out=r_sb[ct], in_=ps)
                    nc.vector.tensor_mul(r_sb[ct], r_sb[ct], rs_sb[ct])

            # ---------- phase C: out = (M + I) x + r ----------
            MMW = 512         # matmul free width (one PSUM bank of f32)
            mult = mybir.AluOpType.mult
            add = mybir.AluOpType.add
            with (
                tc.tile_pool(name="psc", bufs=2, space=MemorySpace.PSUM) as psc,
                tc.tile_pool(name="outp", bufs=3) as outp,
            ):
                hw_ = CW // 2
                for ch in range(NRES):
                    for ct in range(2):
                        ot = outp.tile([P, CW], bf16, tag="ot", name="ot")
                        # half-width PSUM tiles (2 banks x 4 bufs) so the MM
                        # stream rotates into freed banks at a finer grain;
                        # post-op halves run on ACT || DVE concurrently
                        for s in range(2):
                            sl = slice(s * hw_, (s + 1) * hw_)
                            ps = psc.tile([P, hw_], f32, tag="ops",
                                          name="ops", bufs=4)
                            for q in range(hw_ // MMW):
                                qs = ch * CW + s * hw_ + q * MMW
                                for e in range(2):
                                    nc.tensor.matmul(
                                        ps[:, q * MMW:(q + 1) * MMW],
                                        mt_sb[e][:, ct * P:(ct + 1) * P],
                                        xn[e][:, qs:qs + MMW],
                                        start=(e == 0), stop=(e == 1),
                                    )
                            if s % 2 == 0:
                                nc.scalar.activation(
                                    out=ot[:, sl], in_=ps, func=Ident,
                                    bias=r_sb[ct], scale=rs_sb[ct])
                            else:
                                nc.vector.tensor_scalar(
                                    out=ot[:, sl], in0=ps,
                                    scalar1=rs_sb[ct], scalar2=r_sb[ct],
                                    op0=mult, op1=add)
                            if ch == NRES - 1:
                                nc.sync.dma_start(
                                    out=out_ap[ct * P:(ct + 1) * P,
                                               ch * CW + s * hw_:
                                               ch * CW + (s + 1) * hw_],
                                    in_=ot[:, sl])
                        if ch < NRES - 1:
                            nc.sync.dma_start(
                                out=out_ap[ct * P:(ct + 1) * P,
                                           ch * CW:(ch + 1) * CW],
                                in_=ot)

    nc.compile()
    return nc


def _get_nc():
    key = "v2"
    if key not in _BUILD_CACHE:
        _BUILD_CACHE[key] = _build()
    return _BUILD_CACHE[key]


def kernel(x, wq, bq, wk, bk, wv, bv):
    global LAST_RESULT
    from concourse.bass_utils import run_bass_kernel_spmd

    nc = _get_nc()

    x = np.ascontiguousarray(np.asarray(x, dtype=np.float32))
    xf = x.reshape(B, C, N)
    wq = np.ascontiguousarray(np.asarray(wq, dtype=np.float32))
    wk = np.ascontiguousarray(np.asarray(wk, dtype=np.float32))
    wv = np.ascontiguousarray(np.asarray(wv, dtype=np.float32))
    bq = np.ascontiguousarray(np.asarray(bq, dtype=np.float32))
    bk = np.ascontiguousarray(np.asarray(bk, dtype=np.float32))
    bv = np.ascontiguousarray(np.asarray(bv, dtype=np.float32))

    in_maps = _make_in_maps(xf, wq, bq, wk, bk, wv, bv)

    res = run_bass_kernel_spmd(nc, in_maps, core_ids=list(range(NCORES)))
    LAST_RESULT = res

    out = np.empty((B, C, N), dtype=np.float32)
    for i in range(NCORES):
        b, h = i // 2, i % 2
        out[b, :, h * HALF:(h + 1) * HALF] = np.asarray(
            res.results[i]["out"]).astype(np.float32)
    return out.reshape(B, C, W, H)


# ---------------------------------------------------------------------------
# Dev-loop helpers (not used by the grading path)
# ---------------------------------------------------------------------------

def timeline_ns():
    """Cost-model simulated duration of one core's program (ns)."""
    from concourse.timeline_sim import TimelineSim
    nc = _get_nc()
    ts = TimelineSim(nc)
    return ts.simulate()


def _make_in_maps(xf, wq, bq, wk, bk, wv, bv):
    ones_pad = np.zeros((N, 2), dtype=np.float32)
    ones_pad[:, 0] = 1.0
    wqwk = np.ascontiguousarray(np.concatenate([wq, wk], axis=0))
    vpk = np.concatenate(
        [wv, bv[:, None], np.zeros((C, 1), np.float32)],
        axis=1).astype(ml_dtypes.bfloat16)
    bpk = np.concatenate([bq, bk])[None, :].astype(np.float32)
    in_maps = []
    for i in range(NCORES):
        b, h = i // 2, i % 2
        xTb = np.concatenate([xf[b].T, ones_pad], axis=1)
        # own spatial half first: the kernel keeps the first NCHUNK/2 chunks
        # resident and derives its output columns from them
        xt = np.concatenate([xTb[h * HALF:(h + 1) * HALF],
                             xTb[(1 - h) * HALF:(2 - h) * HALF]],
                            axis=0).astype(ml_dtypes.bfloat16)
        xn = np.ascontiguousarray(
            xf[b][:, h * HALF + HALF // 2:(h + 1) * HALF]
        ).astype(ml_dtypes.bfloat16)
        in_maps.append({
            "xt": xt, "xn": xn,
            "wqwk": wqwk, "vpk": vpk, "bpk": bpk,
        })
    return in_maps
